# revision 28
# baseline (speedup 1.0000x reference)
"""Trainium2 Bass kernel for nn_CLNet_5557687681860.

Self-contained 8-core SPMD implementation.  Sharding: 3600 sliding windows
(B=4 x L=900) split 450/core (core c -> batch b=c//2, l-range (c%2)*450).

Per core, in 15 chunks of 30 windows:
  - conv1 computed over the chunk's row range once (windows overlap), plus a
    per-window row-0 fixup and a host-materialized direct path for the 44
    head/tail windows.
  - conv2/3/4 as banded matmuls; even/odd output rows accumulate in separate
    PSUM tiles so 2x2 maxpool is a tensor_tensor max.
  - All data tensors between convs hold elu(x)+1 ("shifted"); 'SAME' pads
    hold 1.0 (memset once).  Each band matrix carries a ones-row bias column
    with per-column edge corrections, so PSUM = pre-activation + bias + 1.
  - Activation per block: t1=max(pA,pB); t2=pairmax(t1); e=exp(t2-1);
    out=max(min(e,1),t2)  (fused scalar_tensor_tensor).
  - conv3/conv4 read the previous layer's output tile directly (band rows
    drop the pads); only conv1->conv2 needs a gather (overlapping windows),
    done as single 4D-AP DMAs via an SBUF row-linear staging tile.
Then fc1/fc2/emb -> z_t (shifted), AllGather inside core pairs, masked
attention (masks folded into q/k as multiplies), and the three heads.
Matmuls run in float32r.
"""

import numpy as np

import concourse.bass as bass
import concourse.bacc as bacc
import concourse.tile as tile
from concourse import mybir
from concourse.bass_utils import run_bass_kernel_spmd

F32 = mybir.dt.float32
F32R = mybir.dt.float32r
AF = mybir.ActivationFunctionType
OP = mybir.AluOpType

L, U, WH, D = 900, 45, 22, 80
NC = 8
W = 450           # windows per core
WI = 406          # interior windows: w in [22,428)
WCH = 30          # windows per chunk; 450 = 15*30
NCH = 15
ZPR = 552         # zp rows: global [l0-23, l0+529)
LG = [113, 113, 113, 111]
MG = [113] * 7 + [109]

_CACHE = {}


# ---------------------------------------------------------------------------
# weight pack layout (static): name -> (nrows, ncols); offsets derived
# ---------------------------------------------------------------------------

def _wpack_layout():
    ents = []
    for i in range(3):
        for v in ("A", "B"):
            ents.append((f"c1s_{i}{v}", 35, 128))
    for i in range(3):
        for v in ("A", "B"):
            ents.append((f"c1d_{i}{v}", 47, 88))
    for i in range(3):
        for v in ("A", "B"):
            ents.append((f"c2_{i}{v}", 89, 88))
    for i in range(3):
        for v in ("A", "B"):
            ents.append((f"c3_{i}{v}", 88, 80))
    for i in range(3):
        ents.append((f"c4_{i}", 80, 80))
    for nm, p in (("b1", 128), ("b1p1", 128), ("b1d", 88), ("b1dp1", 88),
                  ("d3", 80), ("b3", 80), ("b3p1", 80), ("b4", 80),
                  ("b4p1", 80)):
        ents.append((nm, p, 1))
    for x4 in range(5):
        ents.append((f"fc1_{x4}", 81, 128))
    ents.append(("fc2T", 128, 40))
    ents.append(("bfc2", 40, 1))
    ents.append(("wqT", 41, 40))
    ents.append(("wkT", 41, 40))
    ents.append(("wvT", 41, 40))
    ents.append(("h1", 41, 32))
    ents.append(("h2", 31, 3))
    ents.append(("eye", 128, 128))
    off = {}
    c = 0
    for nm, p, w in ents:
        off[nm] = (p, w, c)
        c += w
    return off, c


W_OFF, W_NCOL = _wpack_layout()
# split points for the 3 weight-load DMAs (by column ranges)
W_SPLIT1 = W_OFF["c2_0A"][2]          # c1s + c1d
W_SPLIT2 = W_OFF["fc1_0"][2]          # c2 + c3 + c4

# cpack (per-core): mvb [41,900] | mv_loc [41,450] | embT_aug [41,40]
C_NCOL = 900 + 450 + 40
C_MVB, C_MVL, C_EMB = 0, 900, 1350


# ---------------------------------------------------------------------------
# host-side weight packing
# ---------------------------------------------------------------------------

def _c1s(cw1, dx, variant):
    m = np.zeros((35, 128), np.float32)
    off = 1 if variant == "B" else 0
    for j in range(32):
        for du in (-1, 0, 1):
            p = j + du + 1 + off
            if 0 <= p < 35:
                for oc in range(4):
                    m[p, oc * 32 + j] = cw1[oc, 0, du + 1, dx + 1]
    return m


def _c1d(cw1, dx, variant):
    m = np.zeros((47, 88), np.float32)
    off = 1 if variant == "B" else 0
    for up in range(22):
        for du in (-1, 0, 1):
            p = 2 * up + du + 1 + off
            if 0 <= p < 47:
                for oc in range(4):
                    m[p, oc * 22 + up] = cw1[oc, 0, du + 1, dx + 1]
    return m


def _band(cw, cb, dx, off, n_in, n_out, in_stride2, with_bias):
    """Generic banded lhsT for conv2/3/4.

    cw: [OC, IC, 3, 3]; input rows u in [0, n_in) per ic; output rows
    j in [0, n_out) per oc; tap u = (2j if in_stride2 else j) + du + off.
    Returns [IC*n_in + 1, OC*n_out] with the ones-row bias (bias + 1 -
    sum of present taps over all dx) when with_bias.
    """
    OC, IC = cw.shape[0], cw.shape[1]
    K = IC * n_in
    m = np.zeros((K + (1 if with_bias is not None else 0), OC * n_out),
                 np.float32)
    for j in range(n_out):
        base = 2 * j if in_stride2 else j
        for du in (-1, 0, 1):
            u = base + du + off
            if 0 <= u < n_in:
                for ic in range(IC):
                    for oc in range(OC):
                        m[ic * n_in + u, oc * n_out + j] = cw[oc, ic, du + 1,
                                                              dx + 1]
    if with_bias:
        for j in range(n_out):
            base = 2 * j if in_stride2 else j
            dus = [du for du in (-1, 0, 1) if 0 <= base + du + off < n_in]
            for oc in range(OC):
                pres = cw[oc, :, [du + 1 for du in dus], :].sum()
                m[K, oc * n_out + j] = 1.0 + cb[oc] - pres
    return m


def _edge_bias(cw, cb, off, n_in, n_out, in_stride2):
    """Per-column constant c = cb - sum(present taps) for the given variant
    offset; [OC*n_out] vector."""
    OC = cw.shape[0]
    out = np.zeros(OC * n_out, np.float32)
    for j in range(n_out):
        base = 2 * j if in_stride2 else j
        dus = [du for du in (-1, 0, 1) if 0 <= base + du + off < n_in]
        for oc in range(OC):
            out[oc * n_out + j] = cb[oc] - cw[oc, :, [du + 1 for du in dus],
                                              :].sum()
    return out


def _prep_shared(I):
    cw1, cw2 = np.asarray(I["cw1"], np.float32), np.asarray(I["cw2"], np.float32)
    cw3, cw4 = np.asarray(I["cw3"], np.float32), np.asarray(I["cw4"], np.float32)
    cb1, cb2 = np.asarray(I["cb1"], np.float32), np.asarray(I["cb2"], np.float32)
    cb3, cb4 = np.asarray(I["cb3"], np.float32), np.asarray(I["cb4"], np.float32)

    d = {}
    for i, dx in enumerate((-1, 0, 1)):
        wb = dx == 0
        for v, off in (("A", 0), ("B", 1)):
            d[f"c1s_{i}{v}"] = _c1s(cw1, dx, v)
            d[f"c1d_{i}{v}"] = _c1d(cw1, dx, v)
            d[f"c2_{i}{v}"] = _band(cw2, cb2, dx, off, 22, 11, True, wb)
            d[f"c3_{i}{v}"] = _band(cw3, None, dx, off, 11, 5, True, None)
        d[f"c4_{i}"] = _band(cw4, None, dx, 0, 5, 5, False, None)
    d["b1"] = np.repeat(cb1, 32).reshape(128, 1)
    d["b1p1"] = d["b1"] + 1.0
    d["b1d"] = np.repeat(cb1, 22).reshape(88, 1)
    d["b1dp1"] = d["b1d"] + 1.0
    # conv3: variant B has no dropped taps; cB = cb3 - sum(all taps)
    cB3 = _edge_bias(cw3, cb3, 1, 11, 5, True)
    cA3 = _edge_bias(cw3, cb3, 0, 11, 5, True)
    d["b3"] = cB3.reshape(80, 1)
    d["b3p1"] = d["b3"] + 1.0
    d["d3"] = (cA3 - cB3).reshape(80, 1)
    c4b = _edge_bias(cw4, cb4, 0, 5, 5, False)
    d["b4"] = c4b.reshape(80, 1)
    d["b4p1"] = d["b4"] + 1.0

    fc1w = np.asarray(I["fc1w"], np.float32)
    fc1b = np.asarray(I["fc1b"], np.float32)
    for x4 in range(5):
        m = np.zeros((81, 128), np.float32)
        for oc in range(16):
            for u in range(5):
                m[oc * 5 + u, :] = fc1w[:, oc * 25 + u * 5 + x4]
        if x4 == 0:
            m[80, :] = 1.0 + fc1b - fc1w.sum(1)
        d[f"fc1_{x4}"] = m

    fc2w = np.asarray(I["fc2w"], np.float32)
    d["fc2T"] = fc2w.T.copy()
    d["bfc2"] = (np.asarray(I["fc2b"], np.float32) - fc2w.sum(1)).reshape(40, 1)

    rt = np.sqrt(np.float32(40.0))
    wq, wk, wv = (np.asarray(I[k], np.float32) for k in ("wq", "wk", "wv"))
    d["wqT"] = np.concatenate(
        [wq.T, ((I["bq"] - wq.sum(1)) / rt)[None]], 0)
    d["wqT"][:40] /= rt
    d["wkT"] = np.concatenate([wk.T, (I["bk"] - wk.sum(1))[None]], 0)
    d["wvT"] = np.concatenate([wv.T, (I["bv"] - wv.sum(1))[None]], 0)

    h1 = np.zeros((41, 32), np.float32)
    h2 = np.zeros((31, 3), np.float32)
    for h, (w1k, b1k, w2k, b2k) in enumerate(
        [("flw1", "flb1", "flw2", "flb2"), ("stw1", "stb1", "stw2", "stb2"),
         ("edw1", "edb1", "edw2", "edb2")]):
        h1[:40, h * 10:h * 10 + 10] = I[w1k].T
        h1[40, h * 10:h * 10 + 10] = np.asarray(I[b1k]) + 1.0
        h2[h * 10:h * 10 + 10, h] = I[w2k][0]
        h2[30, h] = I[b2k][0] - np.asarray(I[w2k][0]).sum()
    d["h1"] = h1
    d["h2"] = h2
    d["eye"] = np.eye(128, dtype=np.float32)

    wpack = np.zeros((128, W_NCOL), np.float32)
    for nm, (p, w, c0) in W_OFF.items():
        a = d[nm]
        assert a.shape == (p, w), (nm, a.shape, (p, w))
        wpack[:p, c0:c0 + w] = a
    return wpack


def _prep_core(I, c, wpack):
    b, l0 = c // 2, (c % 2) * W
    z = np.asarray(I["z"], np.float32)[b, 0]
    dur = int(np.asarray(I["dur"]).reshape(-1)[b])

    # zp: row i <-> global row l0-23+i, col j <-> x j-1 (zero padded)
    zp = np.zeros((ZPR, 82), np.float32)
    g0, g1 = max(l0 - 23, 0), min(l0 + ZPR - 23, L)
    zp[g0 - (l0 - 23):g1 - (l0 - 23), 1:81] = z[g0:g1]

    # direct windows, host-materialized: wind[u+1, wd, x+1]
    wind = np.zeros((47, 44, 82), np.float32)
    for wd in range(44):
        w = wd if wd < 22 else wd + WI
        l = l0 + w
        end = min(l + 23, L)
        start = max(l - 22, 0)
        n = end - start
        wind[U - n + 1:U + 1, wd, 1:81] = z[start:end]

    # row-0 fixup, all windows (stored shifted: elu+1); ch 4 = ones row
    cw1 = np.asarray(I["cw1"], np.float32)
    cb1 = np.asarray(I["cb1"], np.float32)
    zp2 = np.zeros((902, 82), np.float32)
    zp2[1:901, 1:81] = z
    c1f = np.zeros((4, 900, 80), np.float32)
    for du in range(3):
        for dx in range(3):
            c1f += cw1[:, 0, du, dx, None, None] * zp2[du:du + 900,
                                                       dx:dx + 80]
    sh = np.zeros((4, 900, 80), np.float32)
    for dx in range(3):
        sh += cw1[:, 0, 0, dx, None, None] * zp2[0:900, dx:dx + 80]
    ss = l0 + np.arange(22, 428) - 22
    r0 = c1f[:, ss, :] - sh[:, ss, :]
    r1 = c1f[:, ss + 1, :]
    row = np.maximum(r0, r1).reshape(4, WI, 40, 2).max(-1)
    row = row + cb1[:, None, None]
    p1fx = np.ones((5, W, 40), np.float32)
    p1fx[:4, 22:428] = (np.where(row > 0, row,
                                 np.exp(np.minimum(row, 0)) - 1) + 1)
    # direct windows: first pooled row from the materialized window
    for wd in range(44):
        w = wd if wd < 22 else wd + WI
        cwin = np.zeros((4, 2, 80), np.float32)
        for du in range(3):
            for dx in range(3):
                for u in range(2):
                    cwin[:, u, :] += (cw1[:, 0, du, dx, None]
                                      * wind[u + du, wd, dx:dx + 80])
        rowd = cwin.max(1).reshape(4, 40, 2).max(-1) + cb1[:, None]
        p1fx[:4, w] = (np.where(rowd > 0, rowd,
                                np.exp(np.minimum(rowd, 0)) - 1) + 1)

    mv = (np.arange(L) < dur).astype(np.float32)
    cpack = np.zeros((41, C_NCOL), np.float32)
    cpack[:, C_MVB:C_MVB + L] = mv[None, :]
    cpack[:, C_MVL:C_MVL + W] = mv[None, l0:l0 + W]
    t = np.asarray(I["targets_onehot"], np.float32)[b]
    embw = np.asarray(I["embw"], np.float32)
    emb_aug = np.zeros((41, 40), np.float32)
    emb_aug[:40] = embw[:, :40].T
    emb_aug[40] = np.asarray(I["embb"], np.float32) + embw[:, 40:] @ t + 1.0
    cpack[:, C_EMB:C_EMB + 40] = emb_aug

    m = {"zp": zp, "wind": wind, "p1fx": p1fx, "wpack": wpack, "cpack": cpack}
    return {k: np.ascontiguousarray(v, np.float32) for k, v in m.items()}


# ---------------------------------------------------------------------------
# device program
# ---------------------------------------------------------------------------

def _build_program():
    nc = bacc.Bacc("TRN2", target_bir_lowering=False, debug=False,
                   num_devices=NC)

    zp_d = nc.dram_tensor("zp", [ZPR, 82], F32, kind="ExternalInput").ap()
    wind_d = nc.dram_tensor("wind", [47, 44, 82], F32,
                            kind="ExternalInput").ap()
    p1fx_d = nc.dram_tensor("p1fx", [5, W, 40], F32,
                            kind="ExternalInput").ap()
    wpack_d = nc.dram_tensor("wpack", [128, W_NCOL], F32,
                             kind="ExternalInput").ap()
    cpack_d = nc.dram_tensor("cpack", [41, C_NCOL], F32,
                             kind="ExternalInput").ap()
    out_d = nc.dram_tensor("out", [3, W], F32, kind="ExternalOutput").ap()

    zt_loc = nc.dram_tensor("zt_loc", [40 * W], F32)
    zt_full = nc.dram_tensor("zt_full", [2, 40 * W], F32)

    import contextlib
    with tile.TileContext(nc) as tc, contextlib.ExitStack() as ctx:
        wp = ctx.enter_context(tc.tile_pool(name="w", bufs=1))
        sb = ctx.enter_context(tc.tile_pool(name="sb", bufs=2))
        pr = ctx.enter_context(tc.tile_pool(name="pr", bufs=1))
        ps = ctx.enter_context(tc.tile_pool(name="ps", bufs=2, space="PSUM"))
        ps1 = ctx.enter_context(tc.tile_pool(name="ps1", bufs=1,
                                             space="PSUM"))
        dr = ctx.enter_context(tc.tile_pool(name="dr", bufs=2, space="DRAM"))

        wt = wp.tile([128, W_NCOL], F32R, tag="wpack")
        nc.sync.dma_start(wt[:, 0:W_SPLIT1],
                          wpack_d[:, 0:W_SPLIT1].bitcast(F32R))
        nc.sync.dma_start(wt[:, W_SPLIT1:W_SPLIT2],
                          wpack_d[:, W_SPLIT1:W_SPLIT2].bitcast(F32R))
        nc.sync.dma_start(wt[:, W_SPLIT2:],
                          wpack_d[:, W_SPLIT2:].bitcast(F32R))
        ct = wp.tile([41, C_NCOL], F32R, tag="cpack")
        nc.sync.dma_start(ct[:], cpack_d.bitcast(F32R))

        def ws(nm):
            p, w, c0 = W_OFF[nm]
            return wt[0:p, c0:c0 + w]

        neg1 = wp.tile([128, 1], F32, tag="neg1")
        nc.gpsimd.memset(neg1[:], -1.0)

        def wb(nm, P):
            return ws(nm)[0:P, :].bitcast(F32)

        feat = pr.tile([81, W, 5], F32R, tag="feat")
        nc.gpsimd.memset(feat[:].bitcast(F32), 1.0)

        def block_ones(nn, pA, pB, out_ap, tagsuf):
            """conv2 style (bias in ones row): t1=max(pA,pB); t2=pairmax;
            e=exp(t2-1); out=max(min(e,1),t2).  HW allows only one PSUM
            operand per instruction, so pA is staged through SBUF."""
            P = pA.shape[0]
            sA = sb.tile([128, 480], F32, tag="b_sA" + tagsuf)
            nc.scalar.activation(sA[0:P, 0:nn], pA, AF.Identity)
            t1 = sb.tile([128, 480], F32, tag="b_t1" + tagsuf)
            nc.vector.tensor_tensor(t1[0:P, 0:nn], sA[0:P, 0:nn], pB, OP.max)
            t2 = sb.tile([128, 240], F32, tag="b_t2" + tagsuf)
            i3 = t1[0:P, 0:nn].rearrange("p (n two) -> p n two", two=2)
            nc.vector.tensor_tensor(t2[0:P, 0:nn // 2].unsqueeze(-1),
                                    i3[:, :, 0:1], i3[:, :, 1:2], OP.max)
            e = sb.tile([128, 240], F32, tag="b_e" + tagsuf)
            nc.scalar.activation(e[0:P, 0:nn // 2], t2[0:P, 0:nn // 2],
                                 AF.Exp, bias=neg1[0:P, :])
            nc.vector.scalar_tensor_tensor(
                out_ap, e[0:P, 0:nn // 2], 1.0, t2[0:P, 0:nn // 2],
                OP.min, OP.max)

        def block_bias(nn, pA, pB, out_ap, tagsuf, b_ap, bp1_ap, d_ap=None):
            """bias-free matmuls: t1=max(pA+d,pB) (d=cA-cB); t2=pairmax;
            e=exp(t2+b); e2=min(e,1); out=max(t2+b+1, e2)."""
            P = pA.shape[0]
            sA = sb.tile([128, 480], F32, tag="b_sA" + tagsuf)
            nc.scalar.activation(sA[0:P, 0:nn], pA, AF.Identity,
                                 bias=(d_ap if d_ap is not None else 0.0))
            t1 = sb.tile([128, 480], F32, tag="b_t1" + tagsuf)
            nc.vector.tensor_tensor(t1[0:P, 0:nn], sA[0:P, 0:nn], pB, OP.max)
            t2 = sb.tile([128, 240], F32, tag="b_t2" + tagsuf)
            i3 = t1[0:P, 0:nn].rearrange("p (n two) -> p n two", two=2)
            nc.vector.tensor_tensor(t2[0:P, 0:nn // 2].unsqueeze(-1),
                                    i3[:, :, 0:1], i3[:, :, 1:2], OP.max)
            e = sb.tile([128, 240], F32, tag="b_e" + tagsuf)
            nc.scalar.activation(e[0:P, 0:nn // 2], t2[0:P, 0:nn // 2],
                                 AF.Exp, bias=b_ap)
            e2 = sb.tile([128, 240], F32, tag="b_e2" + tagsuf)
            nc.vector.tensor_scalar_min(e2[0:P, 0:nn // 2],
                                        e[0:P, 0:nn // 2], 1.0)
            nc.vector.scalar_tensor_tensor(
                out_ap, t2[0:P, 0:nn // 2], bp1_ap, e2[0:P, 0:nn // 2],
                OP.add, OP.max)

        # ============ stage 1: conv stack, super-chunks of 60 windows ======
        # one conv1 128-row block serves a 60-window gather (rows <= 102)
        for si, (wa, ww) in enumerate([(60 * k, 60) for k in range(7)]
                                      + [(420, 30)]):
            # --- conv1 shared over the super-chunk's 128-row block ---
            zrows = sb.tile([35, 4, 82], F32R, tag="zrows")
            nc.scalar.dma_start(
                zrows[:],
                bass.AP(zp_d.tensor, wa * 82,
                        [[82, 35], [32 * 82, 4], [1, 82]]).bitcast(F32R))
            pA = ps1.tile([128, 4, 80], F32, tag="cA")
            pB = ps1.tile([128, 4, 80], F32, tag="cB")
            for i in range(3):
                nc.tensor.matmul(pA[:], ws(f"c1s_{i}A"),
                                 zrows[:, :, i:i + 80],
                                 start=(i == 0), stop=(i == 2))
            for i in range(3):
                nc.tensor.matmul(pB[:], ws(f"c1s_{i}B"),
                                 zrows[:, :, i:i + 80],
                                 start=(i == 0), stop=(i == 2))
            pech = sb.tile([128, 4, 40], F32, tag="pech")
            block_bias(320, pA[:].rearrange("p a b -> p (a b)"),
                       pB[:].rearrange("p a b -> p (a b)"),
                       pech[:].rearrange("p a b -> p (a b)"), "c1",
                       wb("b1", 128), wb("b1p1", 128))

            # --- row128-linear staging in DRAM (row = 32*sub + j) ---
            p1e = dr.tile([4, 128, 40], F32R, tag="p1e")
            for oc in range(4):
                nc.sync.dma_start(
                    bass.AP(p1e[:].tensor, oc * 5120,
                            [[40, 32], [1280, 4], [1, 40]]),
                    bass.AP(pech[:].tensor, oc * 5120,
                            [[160, 32], [40, 4], [1, 40]]).bitcast(F32R))

            # --- conv2 input gather (windows overlap => per-window rows) ---
            c2p = ww * 42
            c2in = sb.tile([89, ww, 42], F32R, tag="c2in")
            nc.gpsimd.memset(c2in[:, :, 0:1].bitcast(F32), 1.0)
            nc.gpsimd.memset(c2in[:, :, 41:42].bitcast(F32), 1.0)
            ia, ib = max(wa, 22), min(wa + ww, 428)
            n = ib - ia
            for ic in range(4):
                nc.sync.dma_start(
                    bass.AP(c2in[:].tensor, ic * 22 * c2p + (ia - wa) * 42 + 1,
                            [[c2p, 22], [42, n], [1, 40]]),
                    bass.AP(p1e[:].tensor, ic * 5120 + (ia - wa) * 40,
                            [[80, 22], [40, n], [1, 40]]))

            # --- direct (head/tail) windows ---
            if si == 0 or si == 7:
                for gi, (s0, nd) in enumerate(((0, 6), (6, 6), (12, 6),
                                               (18, 4))):
                    wd0 = s0 + (0 if si == 0 else 22)
                    wloc = wd0 if si == 0 else wd0 + WI - 420
                    wint = sb.tile([47, 6, 82], F32R, tag="wint")
                    nc.scalar.dma_start(
                        wint[:, 0:nd, :],
                        wind_d[:, wd0:wd0 + nd, :].bitcast(F32R))
                    dA = ps1.tile([88, 6, 80], F32, tag="cA")
                    dB = ps1.tile([88, 6, 80], F32, tag="cB")
                    for i in range(3):
                        nc.tensor.matmul(dA[:, 0:nd, :], ws(f"c1d_{i}A"),
                                         wint[:, 0:nd, i:i + 80],
                                         start=(i == 0), stop=(i == 2))
                    for i in range(3):
                        nc.tensor.matmul(dB[:, 0:nd, :], ws(f"c1d_{i}B"),
                                         wint[:, 0:nd, i:i + 80],
                                         start=(i == 0), stop=(i == 2))
                    dbuf = sb.tile([88, 6, 40], F32, tag="dbuf")
                    block_bias(
                        nd * 80,
                        dA[:].rearrange("p a b -> p (a b)")[:, 0:nd * 80],
                        dB[:].rearrange("p a b -> p (a b)")[:, 0:nd * 80],
                        dbuf[:].rearrange("p a b -> p (a b)")[:, 0:nd * 40],
                        "c1", wb("b1d", 88), wb("b1dp1", 88))
                    nc.sync.dma_start(
                        c2in[0:88, wloc:wloc + nd, 1:41],
                        dbuf[:, 0:nd, :].bitcast(F32R))

            # --- u=0 fixup + ones row for every window of the super-chunk ---
            nc.sync.dma_start(
                bass.AP(c2in[:].tensor, 1,
                        [[22 * c2p, 5], [42, ww], [1, 40]]),
                bass.AP(p1fx_d.tensor, wa * 40,
                        [[W * 40, 5], [40, ww], [1, 40]]).bitcast(F32R))

            # --- conv2: N-chunks of 10 windows ---
            t2e = sb.tile([88, ww, 22], F32R, tag="t2e")
            nc.gpsimd.memset(t2e[:, :, 0:1].bitcast(F32), 1.0)
            nc.gpsimd.memset(t2e[:, :, 21:22].bitcast(F32), 1.0)
            for k in range(ww // 10):
                na = k * 10
                pA2 = ps.tile([88, 10, 40], F32, tag="pA")
                pB2 = ps.tile([88, 10, 40], F32, tag="pB")
                for i in range(3):
                    rhs = c2in[:, na:na + 10, i:i + 40]
                    nc.tensor.matmul(pA2[:], ws(f"c2_{i}A"), rhs,
                                     start=(i == 0), stop=(i == 2))
                for i in range(3):
                    rhs = c2in[:, na:na + 10, i:i + 40]
                    nc.tensor.matmul(pB2[:], ws(f"c2_{i}B"), rhs,
                                     start=(i == 0), stop=(i == 2))
                block_ones(400, pA2[:].rearrange("p a b -> p (a b)"),
                           pB2[:].rearrange("p a b -> p (a b)"),
                           t2e[:, na:na + 10, 1:21], "c2")

            # --- conv3: N-chunks of 15 ---
            t3e = sb.tile([80, ww, 12], F32R, tag="t3e")
            nc.gpsimd.memset(t3e[:, :, 0:1].bitcast(F32), 1.0)
            nc.gpsimd.memset(t3e[:, :, 11:12].bitcast(F32), 1.0)
            for k in range(ww // 15):
                na = k * 15
                pA3 = ps.tile([80, 15, 20], F32, tag="pA")
                pB3 = ps.tile([80, 15, 20], F32, tag="pB")
                for i in range(3):
                    rhs = t2e[:, na:na + 15, i:i + 20]
                    nc.tensor.matmul(pA3[:], ws(f"c3_{i}A"), rhs,
                                     start=(i == 0), stop=(i == 2))
                for i in range(3):
                    rhs = t2e[:, na:na + 15, i:i + 20]
                    nc.tensor.matmul(pB3[:], ws(f"c3_{i}B"), rhs,
                                     start=(i == 0), stop=(i == 2))
                block_bias(300, pA3[:].rearrange("p a b -> p (a b)"),
                           pB3[:].rearrange("p a b -> p (a b)"),
                           t3e[:, na:na + 15, 1:11], "c3",
                           wb("b3", 80), wb("b3p1", 80), wb("d3", 80))

            # --- conv4 (pool 1x2 only), groups of 30 ---
            for k in range(ww // 30):
                na = k * 30
                pC4 = ps1.tile([80, 30, 10], F32, tag="pC")
                for i in range(3):
                    nc.tensor.matmul(pC4[:], ws(f"c4_{i}"),
                                     t3e[:, na:na + 30, i:i + 10],
                                     start=(i == 0), stop=(i == 2))
                s4 = sb.tile([128, 480], F32, tag="b_sAc1")
                nc.scalar.activation(s4[0:80, 0:300],
                                     pC4[:].rearrange("p a b -> p (a b)"),
                                     AF.Identity)
                t2c = sb.tile([128, 240], F32, tag="b_t2c1")
                i3 = s4[0:80, 0:300].rearrange("p (n two) -> p n two", two=2)
                nc.vector.tensor_tensor(t2c[0:80, 0:150].unsqueeze(-1),
                                        i3[:, :, 0:1], i3[:, :, 1:2], OP.max)
                e4 = sb.tile([128, 240], F32, tag="b_ec1")
                nc.scalar.activation(e4[0:80, 0:150], t2c[0:80, 0:150],
                                     AF.Exp, bias=wb("b4", 80))
                e42 = sb.tile([128, 240], F32, tag="b_e2c1")
                nc.vector.tensor_scalar_min(e42[0:80, 0:150],
                                            e4[0:80, 0:150], 1.0)
                nc.vector.scalar_tensor_tensor(
                    feat[0:80, wa + na:wa + na + 30, :].rearrange(
                        "p a b -> p (a b)"),
                    t2c[0:80, 0:150], wb("b4p1", 80), e42[0:80, 0:150],
                    OP.add, OP.max)

        # ============ stage 3: fc1/fc2/emb -> z_t ==========================
        f1 = ps.tile([128, W], F32, tag="pA")
        for x4 in range(5):
            nc.tensor.matmul(f1[:], ws(f"fc1_{x4}"), feat[:, :, x4:x4 + 1],
                             start=(x4 == 0), stop=(x4 == 4))
        ef = sb.tile([128, W], F32, tag="ef")
        nc.scalar.activation(ef[:], f1[:], AF.Exp, bias=neg1[:])
        fc1e = pr.tile([128, W], F32R, tag="fc1e")
        nc.vector.scalar_tensor_tensor(fc1e[:], ef[:], 1.0, f1[:],
                                       OP.min, OP.max)

        zp0 = ps.tile([40, W], F32, tag="pB")
        nc.tensor.matmul(zp0[:], ws("fc2T"), fc1e[:], start=True, stop=True)
        zp0s = pr.tile([41, W], F32R, tag="zp0s")
        nc.gpsimd.memset(zp0s[:].bitcast(F32), 1.0)
        nc.scalar.activation(zp0s[0:40, :], zp0[:], AF.Identity,
                             bias=ws("bfc2").bitcast(F32))

        ztp = ps.tile([40, W], F32, tag="pA")
        nc.tensor.matmul(ztp[:], ct[:, C_EMB:C_EMB + 40], zp0s[:],
                         start=True, stop=True)
        ez = sb.tile([40, W], F32, tag="ef")
        nc.scalar.activation(ez[:], ztp[:], AF.Exp, bias=neg1[0:40, :])
        zt = pr.tile([41, W], F32R, tag="zt")
        nc.gpsimd.memset(zt[:].bitcast(F32), 1.0)
        nc.vector.scalar_tensor_tensor(zt[0:40, :], ez[:], 1.0, ztp[:],
                                       OP.min, OP.max)

        # ============ stage 4: AllGather z_t (shifted) =====================
        nc.sync.dma_start(zt_loc.ap().rearrange("(p f) -> p f", p=40),
                          zt[0:40, :].bitcast(F32))
        nc.gpsimd.collective_compute(
            "AllGather", OP.bypass,
            replica_groups=[[0, 1], [2, 3], [4, 5], [6, 7]],
            ins=[zt_loc.ap()], outs=[zt_full.ap()])
        zta = pr.tile([41, L], F32R, tag="zta")
        nc.gpsimd.memset(zta[:].bitcast(F32), 1.0)
        nc.sync.dma_start(
            zta[0:40, :],
            bass.AP(zt_full.ap().tensor, 0,
                    [[W, 40], [40 * W, 2], [1, W]]).bitcast(F32R))

        # ============ stage 5: attention ===================================
        qp = ps.tile([40, W], F32, tag="pB")
        nc.tensor.matmul(qp[:], ws("wqT"), zt[:], start=True, stop=True)
        q_sb = pr.tile([40, W], F32R, tag="q_sb")
        nc.vector.tensor_tensor(q_sb[:], qp[:],
                                ct[0:40, C_MVL:C_MVL + W], OP.mult)

        k_sb = pr.tile([40, L], F32R, tag="k_sb")
        for h in range(2):
            kp = ps.tile([40, W], F32, tag="pA")
            nc.tensor.matmul(kp[:], ws("wkT"), zta[:, h * W:(h + 1) * W],
                             start=True, stop=True)
            nc.vector.tensor_tensor(k_sb[:, h * W:(h + 1) * W], kp[:],
                                    ct[0:40, C_MVB + h * W:C_MVB + (h + 1) * W],
                                    OP.mult)

        vps = ps1.tile([113, 8, 40], F32, tag="pC")
        m0 = 0
        for mg in range(8):
            nc.tensor.matmul(vps[0:MG[mg], mg:mg + 1, :],
                             zta[:, m0:m0 + MG[mg]], ws("wvT"),
                             start=True, stop=True)
            m0 += MG[mg]
        v_all = pr.tile([113, 8, 40], F32R, tag="v_all")
        nc.scalar.activation(v_all[0:113, 0:7, :], vps[0:113, 0:7, :],
                             AF.Identity)
        nc.scalar.activation(v_all[0:109, 7:8, :], vps[0:109, 7:8, :],
                             AF.Identity)

        ET = pr.tile([113, 8, W], F32R, tag="ET")
        l0g = 0
        for g in range(4):
            lg = LG[g]
            s0 = ps.tile([113, W], F32, tag="pA")
            s1 = ps.tile([113, W], F32, tag="pB")
            nc.tensor.matmul(s0[0:lg, :], q_sb[:, l0g:l0g + lg],
                             k_sb[:, 0:W], start=True, stop=True)
            nc.tensor.matmul(s1[0:lg, :], q_sb[:, l0g:l0g + lg],
                             k_sb[:, W:L], start=True, stop=True)
            E = sb.tile([113, L], F32, tag="E")
            racc = sb.tile([113, 2], F32, tag="racc")
            nc.scalar.activation(E[0:lg, 0:W], s0[0:lg, :], AF.Exp,
                                 accum_out=racc[0:lg, 0:1])
            nc.scalar.activation(E[0:lg, W:L], s1[0:lg, :], AF.Exp,
                                 accum_out=racc[0:lg, 1:2])
            rs = sb.tile([113, 1], F32, tag="rs")
            nc.vector.tensor_tensor(rs[0:lg, :], racc[0:lg, 0:1],
                                    racc[0:lg, 1:2], OP.add)
            rr = sb.tile([113, 1], F32, tag="rr")
            nc.vector.reciprocal(rr[0:lg, :], rs[0:lg, :])
            nc.vector.tensor_scalar_mul(E[0:lg, :], E[0:lg, :], rr[0:lg, 0:1])
            for half in range(2):
                tr = ps1.tile([113, 4, 113], F32, tag="pC")
                for t in range(4):
                    mg = half * 4 + t
                    m0 = 113 * mg
                    nc.tensor.transpose(tr[0:MG[mg], t:t + 1, 0:lg],
                                        E[0:lg, m0:m0 + MG[mg]],
                                        ws("eye")[0:lg, 0:lg].bitcast(F32))
                if half == 0:
                    nc.scalar.activation(
                        ET[0:113, 0:4, l0g:l0g + lg],
                        tr[0:113, 0:4, 0:lg], AF.Identity)
                else:
                    nc.scalar.activation(
                        ET[0:113, 4:7, l0g:l0g + lg],
                        tr[0:113, 0:3, 0:lg], AF.Identity)
                    nc.scalar.activation(
                        ET[0:109, 7:8, l0g:l0g + lg],
                        tr[0:109, 3:4, 0:lg], AF.Identity)
            l0g += lg

        xp_ = ps1.tile([40, W], F32, tag="pC")
        for mg in range(8):
            nc.tensor.matmul(xp_[:], v_all[0:MG[mg], mg:mg + 1, :],
                             ET[0:MG[mg], mg:mg + 1, :], start=(mg == 0),
                             stop=(mg == 7))
        x_aug = pr.tile([41, W], F32R, tag="x_aug")
        nc.gpsimd.memset(x_aug[:].bitcast(F32), 1.0)
        nc.vector.scalar_tensor_tensor(x_aug[0:40, :], zt[0:40, :], -1.0,
                                       xp_[:], OP.add, OP.add)

        # ============ stage 6: heads =======================================
        h1p = ps.tile([32, W], F32, tag="pA")
        nc.tensor.matmul(h1p[:], ws("h1"), x_aug[:], start=True, stop=True)
        eh = sb.tile([32, W], F32, tag="ef")
        nc.scalar.activation(eh[0:30, :], h1p[0:30, :], AF.Exp,
                             bias=neg1[0:30, :])
        h1e = pr.tile([31, W], F32R, tag="h1e")
        nc.gpsimd.memset(h1e[:].bitcast(F32), 1.0)
        nc.vector.scalar_tensor_tensor(h1e[0:30, :], eh[0:30, :], 1.0,
                                       h1p[0:30, :], OP.min, OP.max)
        o3 = ps.tile([3, W], F32, tag="pB")
        nc.tensor.matmul(o3[:], ws("h2"), h1e[:], start=True, stop=True)
        osb = sb.tile([3, W], F32, tag="osb")
        nc.vector.tensor_copy(osb[:], o3[:])
        nc.sync.dma_start(out_d, osb[:])

    nc.compile()
    return nc


def _get_program():
    if "nc" not in _CACHE:
        _CACHE["nc"] = _build_program()
    return _CACHE["nc"]


def kernel(**inputs):
    I = {k: np.asarray(v) for k, v in inputs.items()}
    nc = _get_program()
    wpack = _prep_shared(I)
    in_maps = [_prep_core(I, c, wpack) for c in range(NC)]
    res = run_bass_kernel_spmd(nc, in_maps, list(range(NC))).results
    fl = np.zeros((4, L), np.float32)
    st = np.zeros((4, L), np.float32)
    ed = np.zeros((4, L), np.float32)
    for c in range(NC):
        b, l0 = c // 2, (c % 2) * W
        o = res[c]["out"]
        fl[b, l0:l0 + W] = o[0]
        st[b, l0:l0 + W] = o[1]
        ed[b, l0:l0 + W] = o[2]
    return fl, st, ed


# revision 30
# speedup vs baseline: 1.2585x; 1.2585x over previous
"""Trainium2 Bass kernel for nn_CLNet_5557687681860.

Self-contained 8-core SPMD implementation.  Sharding: 3600 sliding windows
(B=4 x L=900) split 450/core (core c -> batch b=c//2, l-range (c%2)*450).

Per core, in super-chunks of 60 windows (one conv1 128-row block serves two
30-window halves, since a 60-window gather only spans 102 rows):
  - conv1 computed once over the block's 128 consecutive pooled-pair rows
    (windows overlap heavily); a host-computed per-window u=0 fixup plus a
    host-materialized direct path covers the 44 head/tail edge windows.
  - conv2/3/4 as banded matmuls; even/odd output-row variants (A/B)
    accumulate in separate PSUM tiles; pooling is ACT copy + DVE max +
    strided DVE pair-max (HW allows only one PSUM operand per instruction).
  - All data between convs holds elu(x)+1 ("shifted"); 'SAME' pads hold 1.0.
    conv2's bias rides a ones row fed by the fixup DMA (per-column,
    per-variant edge corrections); conv1/3/4 biases are per-partition
    ACT-bias / scalar_tensor_tensor APs:
      e=exp(t2+b); out=max(t2+b+1, min(e,1)) == elu(pool(conv)+bias)+1.
  - conv3/conv4 read the previous layer's output tile directly (band
    matrices drop pad rows); only conv1->conv2 needs a gather (per-window
    overlapping rows), via a row-linear DRAM staging tile (4 stores + 4
    gathers per super-chunk).  DMA queues are split by role (zrows/wind on
    ACT, stores+gathers on SP, pads via Pool memset) to avoid head-of-line
    blocking.
Then fc1/fc2/emb -> z_t (shifted; weight columns absorb the -1), AllGather
inside core pairs, attention with masks folded into q/k as single
multiplies (the reference's -1e-12 masked-fill is numerically equivalent to
0 through softmax), E transposed via PE in 4-packed PSUM tiles, and the
three heads.  Matmuls run in float32r (transposes in f32: the f32r
transpose path fails the HW ISA verifier).
"""

import numpy as np

import concourse.bass as bass
import concourse.bacc as bacc
import concourse.tile as tile
from concourse import mybir
from concourse.bass_utils import run_bass_kernel_spmd

F32 = mybir.dt.float32
F32R = mybir.dt.float32r
AF = mybir.ActivationFunctionType
OP = mybir.AluOpType

L, U, WH, D = 900, 45, 22, 80
NC = 8
W = 450           # windows per core
WI = 406          # interior windows: w in [22,428)
WCH = 30          # windows per chunk; 450 = 15*30
NCH = 15
ZPR = 552         # zp rows: global [l0-23, l0+529)
LG = [113, 113, 113, 111]
MG = [113] * 7 + [109]

_CACHE = {}


# ---------------------------------------------------------------------------
# weight pack layout (static): name -> (nrows, ncols); offsets derived
# ---------------------------------------------------------------------------

def _wpack_layout():
    ents = []
    for i in range(3):
        for v in ("A", "B"):
            ents.append((f"c1s_{i}{v}", 35, 128))
    for i in range(3):
        for v in ("A", "B"):
            ents.append((f"c1d_{i}{v}", 47, 88))
    for i in range(3):
        for v in ("A", "B"):
            ents.append((f"c2_{i}{v}", 89, 88))
    for i in range(3):
        for v in ("A", "B"):
            ents.append((f"c3_{i}{v}", 88, 80))
    for i in range(3):
        ents.append((f"c4_{i}", 80, 80))
    for nm, p in (("b1", 128), ("b1p1", 128), ("b1d", 88), ("b1dp1", 88),
                  ("d3", 80), ("b3", 80), ("b3p1", 80), ("b4", 80),
                  ("b4p1", 80)):
        ents.append((nm, p, 1))
    for x4 in range(5):
        ents.append((f"fc1_{x4}", 81, 128))
    ents.append(("fc2T", 128, 40))
    ents.append(("bfc2", 40, 1))
    ents.append(("wqT", 41, 40))
    ents.append(("wkT", 41, 40))
    ents.append(("wvT", 41, 40))
    ents.append(("h1", 41, 32))
    ents.append(("h2", 31, 3))
    ents.append(("eye", 128, 128))
    off = {}
    c = 0
    for nm, p, w in ents:
        off[nm] = (p, w, c)
        c += w
    return off, c


W_OFF, W_NCOL = _wpack_layout()
# split points for the 3 weight-load DMAs (by column ranges)
W_SPLIT1 = W_OFF["c2_0A"][2]          # c1s + c1d
W_SPLIT2 = W_OFF["fc1_0"][2]          # c2 + c3 + c4

# cpack (per-core): mvb [41,900] | mv_loc [41,450] | embT_aug [41,40]
C_NCOL = 900 + 450 + 40
C_MVB, C_MVL, C_EMB = 0, 900, 1350


# ---------------------------------------------------------------------------
# host-side weight packing
# ---------------------------------------------------------------------------

def _c1s(cw1, dx, variant):
    m = np.zeros((35, 128), np.float32)
    off = 1 if variant == "B" else 0
    for j in range(32):
        for du in (-1, 0, 1):
            p = j + du + 1 + off
            if 0 <= p < 35:
                for oc in range(4):
                    m[p, oc * 32 + j] = cw1[oc, 0, du + 1, dx + 1]
    return m


def _c1d(cw1, dx, variant):
    m = np.zeros((47, 88), np.float32)
    off = 1 if variant == "B" else 0
    for up in range(22):
        for du in (-1, 0, 1):
            p = 2 * up + du + 1 + off
            if 0 <= p < 47:
                for oc in range(4):
                    m[p, oc * 22 + up] = cw1[oc, 0, du + 1, dx + 1]
    return m


def _band(cw, cb, dx, off, n_in, n_out, in_stride2, with_bias):
    """Generic banded lhsT for conv2/3/4.

    cw: [OC, IC, 3, 3]; input rows u in [0, n_in) per ic; output rows
    j in [0, n_out) per oc; tap u = (2j if in_stride2 else j) + du + off.
    Returns [IC*n_in + 1, OC*n_out] with the ones-row bias (bias + 1 -
    sum of present taps over all dx) when with_bias.
    """
    OC, IC = cw.shape[0], cw.shape[1]
    K = IC * n_in
    m = np.zeros((K + (1 if with_bias is not None else 0), OC * n_out),
                 np.float32)
    for j in range(n_out):
        base = 2 * j if in_stride2 else j
        for du in (-1, 0, 1):
            u = base + du + off
            if 0 <= u < n_in:
                for ic in range(IC):
                    for oc in range(OC):
                        m[ic * n_in + u, oc * n_out + j] = cw[oc, ic, du + 1,
                                                              dx + 1]
    if with_bias:
        for j in range(n_out):
            base = 2 * j if in_stride2 else j
            dus = [du for du in (-1, 0, 1) if 0 <= base + du + off < n_in]
            for oc in range(OC):
                pres = cw[oc, :, [du + 1 for du in dus], :].sum()
                m[K, oc * n_out + j] = 1.0 + cb[oc] - pres
    return m


def _edge_bias(cw, cb, off, n_in, n_out, in_stride2):
    """Per-column constant c = cb - sum(present taps) for the given variant
    offset; [OC*n_out] vector."""
    OC = cw.shape[0]
    out = np.zeros(OC * n_out, np.float32)
    for j in range(n_out):
        base = 2 * j if in_stride2 else j
        dus = [du for du in (-1, 0, 1) if 0 <= base + du + off < n_in]
        for oc in range(OC):
            out[oc * n_out + j] = cb[oc] - cw[oc, :, [du + 1 for du in dus],
                                              :].sum()
    return out


def _prep_shared(I):
    cw1, cw2 = np.asarray(I["cw1"], np.float32), np.asarray(I["cw2"], np.float32)
    cw3, cw4 = np.asarray(I["cw3"], np.float32), np.asarray(I["cw4"], np.float32)
    cb1, cb2 = np.asarray(I["cb1"], np.float32), np.asarray(I["cb2"], np.float32)
    cb3, cb4 = np.asarray(I["cb3"], np.float32), np.asarray(I["cb4"], np.float32)

    d = {}
    for i, dx in enumerate((-1, 0, 1)):
        wb = dx == 0
        for v, off in (("A", 0), ("B", 1)):
            d[f"c1s_{i}{v}"] = _c1s(cw1, dx, v)
            d[f"c1d_{i}{v}"] = _c1d(cw1, dx, v)
            d[f"c2_{i}{v}"] = _band(cw2, cb2, dx, off, 22, 11, True, wb)
            d[f"c3_{i}{v}"] = _band(cw3, None, dx, off, 11, 5, True, None)
        d[f"c4_{i}"] = _band(cw4, None, dx, 0, 5, 5, False, None)
    d["b1"] = np.repeat(cb1, 32).reshape(128, 1)
    d["b1p1"] = d["b1"] + 1.0
    d["b1d"] = np.repeat(cb1, 22).reshape(88, 1)
    d["b1dp1"] = d["b1d"] + 1.0
    # conv3: variant B has no dropped taps; cB = cb3 - sum(all taps)
    cB3 = _edge_bias(cw3, cb3, 1, 11, 5, True)
    cA3 = _edge_bias(cw3, cb3, 0, 11, 5, True)
    d["b3"] = cB3.reshape(80, 1)
    d["b3p1"] = d["b3"] + 1.0
    d["d3"] = (cA3 - cB3).reshape(80, 1)
    c4b = _edge_bias(cw4, cb4, 0, 5, 5, False)
    d["b4"] = c4b.reshape(80, 1)
    d["b4p1"] = d["b4"] + 1.0

    fc1w = np.asarray(I["fc1w"], np.float32)
    fc1b = np.asarray(I["fc1b"], np.float32)
    for x4 in range(5):
        m = np.zeros((81, 128), np.float32)
        for oc in range(16):
            for u in range(5):
                m[oc * 5 + u, :] = fc1w[:, oc * 25 + u * 5 + x4]
        if x4 == 0:
            m[80, :] = 1.0 + fc1b - fc1w.sum(1)
        d[f"fc1_{x4}"] = m

    fc2w = np.asarray(I["fc2w"], np.float32)
    d["fc2T"] = fc2w.T.copy()
    d["bfc2"] = (np.asarray(I["fc2b"], np.float32) - fc2w.sum(1)).reshape(40, 1)

    rt = np.sqrt(np.float32(40.0))
    wq, wk, wv = (np.asarray(I[k], np.float32) for k in ("wq", "wk", "wv"))
    d["wqT"] = np.concatenate(
        [wq.T, ((I["bq"] - wq.sum(1)) / rt)[None]], 0)
    d["wqT"][:40] /= rt
    d["wkT"] = np.concatenate([wk.T, (I["bk"] - wk.sum(1))[None]], 0)
    d["wvT"] = np.concatenate([wv.T, (I["bv"] - wv.sum(1))[None]], 0)

    h1 = np.zeros((41, 32), np.float32)
    h2 = np.zeros((31, 3), np.float32)
    for h, (w1k, b1k, w2k, b2k) in enumerate(
        [("flw1", "flb1", "flw2", "flb2"), ("stw1", "stb1", "stw2", "stb2"),
         ("edw1", "edb1", "edw2", "edb2")]):
        h1[:40, h * 10:h * 10 + 10] = I[w1k].T
        h1[40, h * 10:h * 10 + 10] = np.asarray(I[b1k]) + 1.0
        h2[h * 10:h * 10 + 10, h] = I[w2k][0]
        h2[30, h] = I[b2k][0] - np.asarray(I[w2k][0]).sum()
    d["h1"] = h1
    d["h2"] = h2
    d["eye"] = np.eye(128, dtype=np.float32)

    wpack = np.zeros((128, W_NCOL), np.float32)
    for nm, (p, w, c0) in W_OFF.items():
        a = d[nm]
        assert a.shape == (p, w), (nm, a.shape, (p, w))
        wpack[:p, c0:c0 + w] = a
    return wpack


def _prep_core(I, c, wpack):
    b, l0 = c // 2, (c % 2) * W
    z = np.asarray(I["z"], np.float32)[b, 0]
    dur = int(np.asarray(I["dur"]).reshape(-1)[b])

    # zp: row i <-> global row l0-23+i, col j <-> x j-1 (zero padded)
    zp = np.zeros((ZPR, 82), np.float32)
    g0, g1 = max(l0 - 23, 0), min(l0 + ZPR - 23, L)
    zp[g0 - (l0 - 23):g1 - (l0 - 23), 1:81] = z[g0:g1]

    # direct windows, host-materialized: wind[u+1, wd, x+1]
    wind = np.zeros((47, 44, 82), np.float32)
    for wd in range(44):
        w = wd if wd < 22 else wd + WI
        l = l0 + w
        end = min(l + 23, L)
        start = max(l - 22, 0)
        n = end - start
        wind[U - n + 1:U + 1, wd, 1:81] = z[start:end]

    # row-0 fixup, all windows (stored shifted: elu+1); ch 4 = ones row
    cw1 = np.asarray(I["cw1"], np.float32)
    cb1 = np.asarray(I["cb1"], np.float32)
    zp2 = np.zeros((902, 82), np.float32)
    zp2[1:901, 1:81] = z
    c1f = np.zeros((4, 900, 80), np.float32)
    for du in range(3):
        for dx in range(3):
            c1f += cw1[:, 0, du, dx, None, None] * zp2[du:du + 900,
                                                       dx:dx + 80]
    sh = np.zeros((4, 900, 80), np.float32)
    for dx in range(3):
        sh += cw1[:, 0, 0, dx, None, None] * zp2[0:900, dx:dx + 80]
    ss = l0 + np.arange(22, 428) - 22
    r0 = c1f[:, ss, :] - sh[:, ss, :]
    r1 = c1f[:, ss + 1, :]
    row = np.maximum(r0, r1).reshape(4, WI, 40, 2).max(-1)
    row = row + cb1[:, None, None]
    p1fx = np.ones((5, W, 40), np.float32)
    p1fx[:4, 22:428] = (np.where(row > 0, row,
                                 np.exp(np.minimum(row, 0)) - 1) + 1)
    # direct windows: first pooled row from the materialized window
    for wd in range(44):
        w = wd if wd < 22 else wd + WI
        cwin = np.zeros((4, 2, 80), np.float32)
        for du in range(3):
            for dx in range(3):
                for u in range(2):
                    cwin[:, u, :] += (cw1[:, 0, du, dx, None]
                                      * wind[u + du, wd, dx:dx + 80])
        rowd = cwin.max(1).reshape(4, 40, 2).max(-1) + cb1[:, None]
        p1fx[:4, w] = (np.where(rowd > 0, rowd,
                                np.exp(np.minimum(rowd, 0)) - 1) + 1)

    mv = (np.arange(L) < dur).astype(np.float32)
    cpack = np.zeros((41, C_NCOL), np.float32)
    cpack[:, C_MVB:C_MVB + L] = mv[None, :]
    cpack[:, C_MVL:C_MVL + W] = mv[None, l0:l0 + W]
    t = np.asarray(I["targets_onehot"], np.float32)[b]
    embw = np.asarray(I["embw"], np.float32)
    emb_aug = np.zeros((41, 40), np.float32)
    emb_aug[:40] = embw[:, :40].T
    emb_aug[40] = np.asarray(I["embb"], np.float32) + embw[:, 40:] @ t + 1.0
    cpack[:, C_EMB:C_EMB + 40] = emb_aug

    m = {"zp": zp, "wind": wind, "p1fx": p1fx, "wpack": wpack, "cpack": cpack}
    return {k: np.ascontiguousarray(v, np.float32) for k, v in m.items()}


# ---------------------------------------------------------------------------
# device program
# ---------------------------------------------------------------------------

def _build_program():
    nc = bacc.Bacc("TRN2", target_bir_lowering=False, debug=False,
                   num_devices=NC)

    zp_d = nc.dram_tensor("zp", [ZPR, 82], F32, kind="ExternalInput").ap()
    wind_d = nc.dram_tensor("wind", [47, 44, 82], F32,
                            kind="ExternalInput").ap()
    p1fx_d = nc.dram_tensor("p1fx", [5, W, 40], F32,
                            kind="ExternalInput").ap()
    wpack_d = nc.dram_tensor("wpack", [128, W_NCOL], F32,
                             kind="ExternalInput").ap()
    cpack_d = nc.dram_tensor("cpack", [41, C_NCOL], F32,
                             kind="ExternalInput").ap()
    out_d = nc.dram_tensor("out", [3, W], F32, kind="ExternalOutput").ap()

    zt_loc = nc.dram_tensor("zt_loc", [40 * W], F32)
    zt_full = nc.dram_tensor("zt_full", [2, 40 * W], F32)

    import contextlib
    with tile.TileContext(nc) as tc, contextlib.ExitStack() as ctx:
        wp = ctx.enter_context(tc.tile_pool(name="w", bufs=1))
        sb = ctx.enter_context(tc.tile_pool(name="sb", bufs=2))
        pr = ctx.enter_context(tc.tile_pool(name="pr", bufs=1))
        ps = ctx.enter_context(tc.tile_pool(name="ps", bufs=2, space="PSUM"))
        ps1 = ctx.enter_context(tc.tile_pool(name="ps1", bufs=1,
                                             space="PSUM"))
        dr = ctx.enter_context(tc.tile_pool(name="dr", bufs=2, space="DRAM"))

        wt = wp.tile([128, W_NCOL], F32R, tag="wpack")
        nc.sync.dma_start(wt[:, 0:W_SPLIT1],
                          wpack_d[:, 0:W_SPLIT1].bitcast(F32R))
        nc.sync.dma_start(wt[:, W_SPLIT1:W_SPLIT2],
                          wpack_d[:, W_SPLIT1:W_SPLIT2].bitcast(F32R))
        nc.sync.dma_start(wt[:, W_SPLIT2:],
                          wpack_d[:, W_SPLIT2:].bitcast(F32R))
        ct = wp.tile([41, C_NCOL], F32R, tag="cpack")
        nc.sync.dma_start(ct[:], cpack_d.bitcast(F32R))

        def ws(nm):
            p, w, c0 = W_OFF[nm]
            return wt[0:p, c0:c0 + w]

        neg1 = wp.tile([128, 1], F32, tag="neg1")
        nc.gpsimd.memset(neg1[:], -1.0)

        def wb(nm, P):
            return ws(nm)[0:P, :].bitcast(F32)

        feat = pr.tile([81, W, 5], F32R, tag="feat")
        nc.gpsimd.memset(feat[:].bitcast(F32), 1.0)

        def block_ones(nn, pA, pB, out_ap, tagsuf):
            """conv2 style (bias in ones row): t1=max(pA,pB); t2=pairmax;
            e=exp(t2-1); out=max(min(e,1),t2).  HW allows only one PSUM
            operand per instruction, so pA is staged through SBUF."""
            P = pA.shape[0]
            sA = sb.tile([128, 480], F32, tag="b_sA" + tagsuf)
            nc.scalar.activation(sA[0:P, 0:nn], pA, AF.Identity)
            t1 = sb.tile([128, 480], F32, tag="b_t1" + tagsuf)
            nc.vector.tensor_tensor(t1[0:P, 0:nn], sA[0:P, 0:nn], pB, OP.max)
            t2 = sb.tile([128, 240], F32, tag="b_t2" + tagsuf)
            i3 = t1[0:P, 0:nn].rearrange("p (n two) -> p n two", two=2)
            nc.vector.tensor_tensor(t2[0:P, 0:nn // 2].unsqueeze(-1),
                                    i3[:, :, 0:1], i3[:, :, 1:2], OP.max)
            e = sb.tile([128, 240], F32, tag="b_e" + tagsuf)
            nc.scalar.activation(e[0:P, 0:nn // 2], t2[0:P, 0:nn // 2],
                                 AF.Exp, bias=neg1[0:P, :])
            nc.vector.scalar_tensor_tensor(
                out_ap, e[0:P, 0:nn // 2], 1.0, t2[0:P, 0:nn // 2],
                OP.min, OP.max)

        def block_bias(nn, pA, pB, out_ap, tagsuf, b_ap, bp1_ap, d_ap=None):
            """bias-free matmuls: t1=max(pA+d,pB) (d=cA-cB); t2=pairmax;
            e=exp(t2+b); e2=min(e,1); out=max(t2+b+1, e2)."""
            P = pA.shape[0]
            sA = sb.tile([128, 480], F32, tag="b_sA" + tagsuf)
            nc.scalar.activation(sA[0:P, 0:nn], pA, AF.Identity,
                                 bias=(d_ap if d_ap is not None else 0.0))
            t1 = sb.tile([128, 480], F32, tag="b_t1" + tagsuf)
            nc.vector.tensor_tensor(t1[0:P, 0:nn], sA[0:P, 0:nn], pB, OP.max)
            t2 = sb.tile([128, 240], F32, tag="b_t2" + tagsuf)
            i3 = t1[0:P, 0:nn].rearrange("p (n two) -> p n two", two=2)
            nc.vector.tensor_tensor(t2[0:P, 0:nn // 2].unsqueeze(-1),
                                    i3[:, :, 0:1], i3[:, :, 1:2], OP.max)
            e = sb.tile([128, 240], F32, tag="b_e" + tagsuf)
            nc.scalar.activation(e[0:P, 0:nn // 2], t2[0:P, 0:nn // 2],
                                 AF.Exp, bias=b_ap)
            e2 = sb.tile([128, 240], F32, tag="b_e2" + tagsuf)
            nc.vector.tensor_scalar_min(e2[0:P, 0:nn // 2],
                                        e[0:P, 0:nn // 2], 1.0)
            nc.vector.scalar_tensor_tensor(
                out_ap, t2[0:P, 0:nn // 2], bp1_ap, e2[0:P, 0:nn // 2],
                OP.add, OP.max)

        # ============ stage 1: conv stack, super-chunks of 60 windows ======
        # one conv1 128-row block serves a 60-window gather (rows <= 102).
        # Software-pipelined: the conv1 + store/gather DMA chain for
        # super-chunk si+1 is emitted before conv2-4 of si, so the ~12us
        # DMA chain overlaps compute instead of idling the PE.
        SCS = [(60 * k, 60) for k in range(7)] + [(420, 30)]

        def conv1_chain(si):
            wa, ww = SCS[si]
            # --- conv1 shared over the super-chunk's 128-row block ---
            zrows = sb.tile([35, 4, 82], F32R, tag="zrows")
            nc.scalar.dma_start(
                zrows[:],
                bass.AP(zp_d.tensor, wa * 82,
                        [[82, 35], [32 * 82, 4], [1, 82]]).bitcast(F32R))
            pA = ps1.tile([128, 4, 80], F32, tag="cA")
            pB = ps1.tile([128, 4, 80], F32, tag="cB")
            for i in range(3):
                nc.tensor.matmul(pA[:], ws(f"c1s_{i}A"),
                                 zrows[:, :, i:i + 80],
                                 start=(i == 0), stop=(i == 2))
            for i in range(3):
                nc.tensor.matmul(pB[:], ws(f"c1s_{i}B"),
                                 zrows[:, :, i:i + 80],
                                 start=(i == 0), stop=(i == 2))
            pech = sb.tile([128, 4, 40], F32, tag="pech")
            block_bias(320, pA[:].rearrange("p a b -> p (a b)"),
                       pB[:].rearrange("p a b -> p (a b)"),
                       pech[:].rearrange("p a b -> p (a b)"), "c1",
                       wb("b1", 128), wb("b1p1", 128))

            # --- row128-linear staging in DRAM (row = 32*sub + j) ---
            p1e = dr.tile([4, 128, 40], F32R, tag="p1e")
            for oc in range(4):
                nc.sync.dma_start(
                    bass.AP(p1e[:].tensor, oc * 5120,
                            [[40, 32], [1280, 4], [1, 40]]),
                    bass.AP(pech[:].tensor, oc * 5120,
                            [[160, 32], [40, 4], [1, 40]]).bitcast(F32R))

            # --- conv2 input gather (windows overlap => per-window rows) ---
            c2p = ww * 42
            c2in = sb.tile([89, ww, 42], F32R, tag="c2in")
            nc.gpsimd.memset(c2in[:, :, 0:1].bitcast(F32), 1.0)
            nc.gpsimd.memset(c2in[:, :, 41:42].bitcast(F32), 1.0)
            ia, ib = max(wa, 22), min(wa + ww, 428)
            n = ib - ia
            for ic in range(4):
                nc.sync.dma_start(
                    bass.AP(c2in[:].tensor, ic * 22 * c2p + (ia - wa) * 42 + 1,
                            [[c2p, 22], [42, n], [1, 40]]),
                    bass.AP(p1e[:].tensor, ic * 5120 + (ia - wa) * 40,
                            [[80, 22], [40, n], [1, 40]]))

            # --- direct (head/tail) windows ---
            if si == 0 or si == 7:
                for gi, (s0, nd) in enumerate(((0, 6), (6, 6), (12, 6),
                                               (18, 4))):
                    wd0 = s0 + (0 if si == 0 else 22)
                    wloc = wd0 if si == 0 else wd0 + WI - 420
                    wint = sb.tile([47, 6, 82], F32R, tag="wint")
                    nc.scalar.dma_start(
                        wint[:, 0:nd, :],
                        wind_d[:, wd0:wd0 + nd, :].bitcast(F32R))
                    dA = ps1.tile([88, 6, 80], F32, tag="cA")
                    dB = ps1.tile([88, 6, 80], F32, tag="cB")
                    for i in range(3):
                        nc.tensor.matmul(dA[:, 0:nd, :], ws(f"c1d_{i}A"),
                                         wint[:, 0:nd, i:i + 80],
                                         start=(i == 0), stop=(i == 2))
                    for i in range(3):
                        nc.tensor.matmul(dB[:, 0:nd, :], ws(f"c1d_{i}B"),
                                         wint[:, 0:nd, i:i + 80],
                                         start=(i == 0), stop=(i == 2))
                    dbuf = sb.tile([88, 6, 40], F32, tag="dbuf")
                    block_bias(
                        nd * 80,
                        dA[:].rearrange("p a b -> p (a b)")[:, 0:nd * 80],
                        dB[:].rearrange("p a b -> p (a b)")[:, 0:nd * 80],
                        dbuf[:].rearrange("p a b -> p (a b)")[:, 0:nd * 40],
                        "c1", wb("b1d", 88), wb("b1dp1", 88))
                    nc.sync.dma_start(
                        c2in[0:88, wloc:wloc + nd, 1:41],
                        dbuf[:, 0:nd, :].bitcast(F32R))

            # --- u=0 fixup + ones row for every window of the super-chunk ---
            nc.sync.dma_start(
                bass.AP(c2in[:].tensor, 1,
                        [[22 * c2p, 5], [42, ww], [1, 40]]),
                bass.AP(p1fx_d.tensor, wa * 40,
                        [[W * 40, 5], [40, ww], [1, 40]]).bitcast(F32R))

            return c2in

        def conv234(si, c2in):
            wa, ww = SCS[si]
            # --- conv2: N-chunks of 10 windows ---
            t2e = sb.tile([88, ww, 22], F32R, tag="t2e")
            nc.gpsimd.memset(t2e[:, :, 0:1].bitcast(F32), 1.0)
            nc.gpsimd.memset(t2e[:, :, 21:22].bitcast(F32), 1.0)
            for k in range(ww // 10):
                na = k * 10
                pA2 = ps.tile([88, 10, 40], F32, tag="pA")
                pB2 = ps.tile([88, 10, 40], F32, tag="pB")
                for i in range(3):
                    rhs = c2in[:, na:na + 10, i:i + 40]
                    nc.tensor.matmul(pA2[:], ws(f"c2_{i}A"), rhs,
                                     start=(i == 0), stop=(i == 2))
                for i in range(3):
                    rhs = c2in[:, na:na + 10, i:i + 40]
                    nc.tensor.matmul(pB2[:], ws(f"c2_{i}B"), rhs,
                                     start=(i == 0), stop=(i == 2))
                block_ones(400, pA2[:].rearrange("p a b -> p (a b)"),
                           pB2[:].rearrange("p a b -> p (a b)"),
                           t2e[:, na:na + 10, 1:21], "c2")

            # --- conv3: N-chunks of 15 ---
            t3e = sb.tile([80, ww, 12], F32R, tag="t3e")
            nc.gpsimd.memset(t3e[:, :, 0:1].bitcast(F32), 1.0)
            nc.gpsimd.memset(t3e[:, :, 11:12].bitcast(F32), 1.0)
            for k in range(ww // 15):
                na = k * 15
                pA3 = ps.tile([80, 15, 20], F32, tag="pA")
                pB3 = ps.tile([80, 15, 20], F32, tag="pB")
                for i in range(3):
                    rhs = t2e[:, na:na + 15, i:i + 20]
                    nc.tensor.matmul(pA3[:], ws(f"c3_{i}A"), rhs,
                                     start=(i == 0), stop=(i == 2))
                for i in range(3):
                    rhs = t2e[:, na:na + 15, i:i + 20]
                    nc.tensor.matmul(pB3[:], ws(f"c3_{i}B"), rhs,
                                     start=(i == 0), stop=(i == 2))
                block_bias(300, pA3[:].rearrange("p a b -> p (a b)"),
                           pB3[:].rearrange("p a b -> p (a b)"),
                           t3e[:, na:na + 15, 1:11], "c3",
                           wb("b3", 80), wb("b3p1", 80), wb("d3", 80))

            # --- conv4 (pool 1x2 only), groups of 30 ---
            for k in range(ww // 30):
                na = k * 30
                pC4 = ps1.tile([80, 30, 10], F32, tag="pC")
                for i in range(3):
                    nc.tensor.matmul(pC4[:], ws(f"c4_{i}"),
                                     t3e[:, na:na + 30, i:i + 10],
                                     start=(i == 0), stop=(i == 2))
                s4 = sb.tile([128, 480], F32, tag="b_sAc1")
                nc.scalar.activation(s4[0:80, 0:300],
                                     pC4[:].rearrange("p a b -> p (a b)"),
                                     AF.Identity)
                t2c = sb.tile([128, 240], F32, tag="b_t2c1")
                i3 = s4[0:80, 0:300].rearrange("p (n two) -> p n two", two=2)
                nc.vector.tensor_tensor(t2c[0:80, 0:150].unsqueeze(-1),
                                        i3[:, :, 0:1], i3[:, :, 1:2], OP.max)
                e4 = sb.tile([128, 240], F32, tag="b_ec1")
                nc.scalar.activation(e4[0:80, 0:150], t2c[0:80, 0:150],
                                     AF.Exp, bias=wb("b4", 80))
                e42 = sb.tile([128, 240], F32, tag="b_e2c1")
                nc.vector.tensor_scalar_min(e42[0:80, 0:150],
                                            e4[0:80, 0:150], 1.0)
                nc.vector.scalar_tensor_tensor(
                    feat[0:80, wa + na:wa + na + 30, :].rearrange(
                        "p a b -> p (a b)"),
                    t2c[0:80, 0:150], wb("b4p1", 80), e42[0:80, 0:150],
                    OP.add, OP.max)

        pend = conv1_chain(0)
        for si in range(len(SCS)):
            nxt = conv1_chain(si + 1) if si + 1 < len(SCS) else None
            conv234(si, pend)
            pend = nxt

        # ============ stage 3: fc1/fc2/emb -> z_t ==========================
        f1 = ps.tile([128, W], F32, tag="pA")
        for x4 in range(5):
            nc.tensor.matmul(f1[:], ws(f"fc1_{x4}"), feat[:, :, x4:x4 + 1],
                             start=(x4 == 0), stop=(x4 == 4))
        ef = sb.tile([128, W], F32, tag="ef")
        nc.scalar.activation(ef[:], f1[:], AF.Exp, bias=neg1[:])
        fc1e = pr.tile([128, W], F32R, tag="fc1e")
        nc.vector.scalar_tensor_tensor(fc1e[:], ef[:], 1.0, f1[:],
                                       OP.min, OP.max)

        zp0 = ps.tile([40, W], F32, tag="pB")
        nc.tensor.matmul(zp0[:], ws("fc2T"), fc1e[:], start=True, stop=True)
        zp0s = pr.tile([41, W], F32R, tag="zp0s")
        nc.gpsimd.memset(zp0s[:].bitcast(F32), 1.0)
        nc.scalar.activation(zp0s[0:40, :], zp0[:], AF.Identity,
                             bias=ws("bfc2").bitcast(F32))

        ztp = ps.tile([40, W], F32, tag="pA")
        nc.tensor.matmul(ztp[:], ct[:, C_EMB:C_EMB + 40], zp0s[:],
                         start=True, stop=True)
        ez = sb.tile([40, W], F32, tag="ef")
        nc.scalar.activation(ez[:], ztp[:], AF.Exp, bias=neg1[0:40, :])
        zt = pr.tile([41, W], F32R, tag="zt")
        nc.gpsimd.memset(zt[:].bitcast(F32), 1.0)
        nc.vector.scalar_tensor_tensor(zt[0:40, :], ez[:], 1.0, ztp[:],
                                       OP.min, OP.max)

        # ============ stage 4: AllGather z_t (shifted) =====================
        nc.sync.dma_start(zt_loc.ap().rearrange("(p f) -> p f", p=40),
                          zt[0:40, :].bitcast(F32))
        nc.gpsimd.collective_compute(
            "AllGather", OP.bypass,
            replica_groups=[[0, 1], [2, 3], [4, 5], [6, 7]],
            ins=[zt_loc.ap()], outs=[zt_full.ap()])
        zta = pr.tile([41, L], F32R, tag="zta")
        nc.gpsimd.memset(zta[:].bitcast(F32), 1.0)
        nc.sync.dma_start(
            zta[0:40, :],
            bass.AP(zt_full.ap().tensor, 0,
                    [[W, 40], [40 * W, 2], [1, W]]).bitcast(F32R))

        # ============ stage 5: attention ===================================
        qp = ps.tile([40, W], F32, tag="pB")
        nc.tensor.matmul(qp[:], ws("wqT"), zt[:], start=True, stop=True)
        q_sb = pr.tile([40, W], F32R, tag="q_sb")
        nc.vector.tensor_tensor(q_sb[:], qp[:],
                                ct[0:40, C_MVL:C_MVL + W], OP.mult)

        k_sb = pr.tile([40, L], F32R, tag="k_sb")
        for h in range(2):
            kp = ps.tile([40, W], F32, tag="pA")
            nc.tensor.matmul(kp[:], ws("wkT"), zta[:, h * W:(h + 1) * W],
                             start=True, stop=True)
            nc.vector.tensor_tensor(k_sb[:, h * W:(h + 1) * W], kp[:],
                                    ct[0:40, C_MVB + h * W:C_MVB + (h + 1) * W],
                                    OP.mult)

        vps = ps1.tile([113, 8, 40], F32, tag="pC")
        m0 = 0
        for mg in range(8):
            nc.tensor.matmul(vps[0:MG[mg], mg:mg + 1, :],
                             zta[:, m0:m0 + MG[mg]], ws("wvT"),
                             start=True, stop=True)
            m0 += MG[mg]
        v_all = pr.tile([113, 8, 40], F32R, tag="v_all")
        nc.scalar.activation(v_all[0:113, 0:7, :], vps[0:113, 0:7, :],
                             AF.Identity)
        nc.scalar.activation(v_all[0:109, 7:8, :], vps[0:109, 7:8, :],
                             AF.Identity)

        ET = pr.tile([113, 8, W], F32R, tag="ET")
        l0g = 0
        for g in range(4):
            lg = LG[g]
            s0 = ps.tile([113, W], F32, tag="pA")
            s1 = ps.tile([113, W], F32, tag="pB")
            nc.tensor.matmul(s0[0:lg, :], q_sb[:, l0g:l0g + lg],
                             k_sb[:, 0:W], start=True, stop=True)
            nc.tensor.matmul(s1[0:lg, :], q_sb[:, l0g:l0g + lg],
                             k_sb[:, W:L], start=True, stop=True)
            E = sb.tile([113, L], F32, tag="E")
            racc = sb.tile([113, 2], F32, tag="racc")
            nc.scalar.activation(E[0:lg, 0:W], s0[0:lg, :], AF.Exp,
                                 accum_out=racc[0:lg, 0:1])
            nc.scalar.activation(E[0:lg, W:L], s1[0:lg, :], AF.Exp,
                                 accum_out=racc[0:lg, 1:2])
            rs = sb.tile([113, 1], F32, tag="rs")
            nc.vector.tensor_tensor(rs[0:lg, :], racc[0:lg, 0:1],
                                    racc[0:lg, 1:2], OP.add)
            rr = sb.tile([113, 1], F32, tag="rr")
            nc.vector.reciprocal(rr[0:lg, :], rs[0:lg, :])
            nc.vector.tensor_scalar_mul(E[0:lg, :], E[0:lg, :], rr[0:lg, 0:1])
            for half in range(2):
                tr = ps1.tile([113, 4, 113], F32, tag="pC")
                for t in range(4):
                    mg = half * 4 + t
                    m0 = 113 * mg
                    nc.tensor.transpose(tr[0:MG[mg], t:t + 1, 0:lg],
                                        E[0:lg, m0:m0 + MG[mg]],
                                        ws("eye")[0:lg, 0:lg].bitcast(F32))
                if half == 0:
                    nc.scalar.activation(
                        ET[0:113, 0:4, l0g:l0g + lg],
                        tr[0:113, 0:4, 0:lg], AF.Identity)
                else:
                    nc.scalar.activation(
                        ET[0:113, 4:7, l0g:l0g + lg],
                        tr[0:113, 0:3, 0:lg], AF.Identity)
                    nc.scalar.activation(
                        ET[0:109, 7:8, l0g:l0g + lg],
                        tr[0:109, 3:4, 0:lg], AF.Identity)
            l0g += lg

        xp_ = ps1.tile([40, W], F32, tag="pC")
        for mg in range(8):
            nc.tensor.matmul(xp_[:], v_all[0:MG[mg], mg:mg + 1, :],
                             ET[0:MG[mg], mg:mg + 1, :], start=(mg == 0),
                             stop=(mg == 7))
        x_aug = pr.tile([41, W], F32R, tag="x_aug")
        nc.gpsimd.memset(x_aug[:].bitcast(F32), 1.0)
        nc.vector.scalar_tensor_tensor(x_aug[0:40, :], zt[0:40, :], -1.0,
                                       xp_[:], OP.add, OP.add)

        # ============ stage 6: heads =======================================
        h1p = ps.tile([32, W], F32, tag="pA")
        nc.tensor.matmul(h1p[:], ws("h1"), x_aug[:], start=True, stop=True)
        eh = sb.tile([32, W], F32, tag="ef")
        nc.scalar.activation(eh[0:30, :], h1p[0:30, :], AF.Exp,
                             bias=neg1[0:30, :])
        h1e = pr.tile([31, W], F32R, tag="h1e")
        nc.gpsimd.memset(h1e[:].bitcast(F32), 1.0)
        nc.vector.scalar_tensor_tensor(h1e[0:30, :], eh[0:30, :], 1.0,
                                       h1p[0:30, :], OP.min, OP.max)
        o3 = ps.tile([3, W], F32, tag="pB")
        nc.tensor.matmul(o3[:], ws("h2"), h1e[:], start=True, stop=True)
        osb = sb.tile([3, W], F32, tag="osb")
        nc.vector.tensor_copy(osb[:], o3[:])
        nc.sync.dma_start(out_d, osb[:])

    nc.compile()
    return nc


def _get_program():
    if "nc" not in _CACHE:
        _CACHE["nc"] = _build_program()
    return _CACHE["nc"]


def kernel(**inputs):
    I = {k: np.asarray(v) for k, v in inputs.items()}
    nc = _get_program()
    wpack = _prep_shared(I)
    in_maps = [_prep_core(I, c, wpack) for c in range(NC)]
    res = run_bass_kernel_spmd(nc, in_maps, list(range(NC))).results
    fl = np.zeros((4, L), np.float32)
    st = np.zeros((4, L), np.float32)
    ed = np.zeros((4, L), np.float32)
    for c in range(NC):
        b, l0 = c // 2, (c % 2) * W
        o = res[c]["out"]
        fl[b, l0:l0 + W] = o[0]
        st[b, l0:l0 + W] = o[1]
        ed[b, l0:l0 + W] = o[2]
    return fl, st, ed


# revision 31
# speedup vs baseline: 1.5205x; 1.2082x over previous
"""Trainium2 Bass kernel for nn_CLNet_5557687681860.

Self-contained 8-core SPMD implementation.  Sharding: 3600 sliding windows
(B=4 x L=900) split 450/core (core c -> batch b=c//2, l-range (c%2)*450).

Per core, in super-chunks of 60 windows (one conv1 128-row block serves two
30-window halves, since a 60-window gather only spans 102 rows):
  - conv1 computed once over the block's 128 consecutive pooled-pair rows
    (windows overlap heavily); a host-computed per-window u=0 fixup plus a
    host-materialized direct path covers the 44 head/tail edge windows.
  - conv2/3/4 as banded matmuls; even/odd output-row variants (A/B)
    accumulate in separate PSUM tiles; pooling is ACT copy + DVE max +
    strided DVE pair-max (HW allows only one PSUM operand per instruction).
  - All data between convs holds elu(x)+1 ("shifted"); 'SAME' pads hold 1.0.
    conv2's bias rides a ones row fed by the fixup DMA (per-column,
    per-variant edge corrections); conv1/3/4 biases are per-partition
    ACT-bias / scalar_tensor_tensor APs:
      e=exp(t2+b); out=max(t2+b+1, min(e,1)) == elu(pool(conv)+bias)+1.
  - conv3/conv4 read the previous layer's output tile directly (band
    matrices drop pad rows); only conv1->conv2 needs a gather (per-window
    overlapping rows), via a row-linear DRAM staging tile (4 stores + 4
    gathers per super-chunk).  DMA queues are split by role (zrows/wind on
    ACT, stores+gathers on SP, pads via Pool memset) to avoid head-of-line
    blocking.
Then fc1/fc2/emb -> z_t (shifted; weight columns absorb the -1), AllGather
inside core pairs, attention with masks folded into q/k as single
multiplies (the reference's -1e-12 masked-fill is numerically equivalent to
0 through softmax), E transposed via PE in 4-packed PSUM tiles, and the
three heads.  Matmuls run in float32r (transposes in f32: the f32r
transpose path fails the HW ISA verifier).
"""

import numpy as np

import concourse.bass as bass
import concourse.bacc as bacc
import concourse.tile as tile
from concourse import mybir
from concourse.bass_utils import run_bass_kernel_spmd

F32 = mybir.dt.float32
F32R = mybir.dt.float32r
AF = mybir.ActivationFunctionType
OP = mybir.AluOpType

L, U, WH, D = 900, 45, 22, 80
NC = 8
W = 450           # windows per core
WI = 406          # interior windows: w in [22,428)
WCH = 30          # windows per chunk; 450 = 15*30
NCH = 15
ZPR = 552         # zp rows: global [l0-23, l0+529)
LG = [113, 113, 113, 111]
MG = [113] * 7 + [109]

_CACHE = {}


# ---------------------------------------------------------------------------
# weight pack layout (static): name -> (nrows, ncols); offsets derived
# ---------------------------------------------------------------------------

def _wpack_layout():
    ents = []
    for i in range(3):
        for v in ("A", "B"):
            ents.append((f"c1s_{i}{v}", 35, 128))
    for i in range(3):
        for v in ("A", "B"):
            ents.append((f"c1d_{i}{v}", 47, 88))
    for i in range(3):
        for v in ("A", "B"):
            ents.append((f"c2_{i}{v}", 89, 88))
    for i in range(3):
        for v in ("A", "B"):
            ents.append((f"c3_{i}{v}", 88, 80))
    for i in range(3):
        ents.append((f"c4_{i}", 80, 80))
    for nm, p in (("b1", 128), ("b1p1", 128), ("b1d", 88), ("b1dp1", 88),
                  ("d3", 80), ("b3", 80), ("b3p1", 80), ("b4", 80),
                  ("b4p1", 80)):
        ents.append((nm, p, 1))
    for x4 in range(5):
        ents.append((f"fc1_{x4}", 81, 128))
    ents.append(("fc2T", 128, 40))
    ents.append(("bfc2", 40, 1))
    ents.append(("wqT", 41, 40))
    ents.append(("wkT", 41, 40))
    ents.append(("wvT", 41, 40))
    ents.append(("h1", 41, 32))
    ents.append(("h2", 31, 3))
    ents.append(("eye", 128, 128))
    off = {}
    c = 0
    for nm, p, w in ents:
        off[nm] = (p, w, c)
        c += w
    return off, c


W_OFF, W_NCOL = _wpack_layout()
# split points for the 3 weight-load DMAs (by column ranges)
W_SPLIT1 = W_OFF["c2_0A"][2]          # c1s + c1d
W_SPLIT2 = W_OFF["fc1_0"][2]          # c2 + c3 + c4

# cpack (per-core): mvb [41,900] | mv_loc [41,450] | embT_aug [41,40]
C_NCOL = 900 + 450 + 40
C_MVB, C_MVL, C_EMB = 0, 900, 1350


# ---------------------------------------------------------------------------
# host-side weight packing
# ---------------------------------------------------------------------------

def _c1s(cw1, dx, variant):
    m = np.zeros((35, 128), np.float32)
    off = 1 if variant == "B" else 0
    for j in range(32):
        for du in (-1, 0, 1):
            p = j + du + 1 + off
            if 0 <= p < 35:
                for oc in range(4):
                    m[p, oc * 32 + j] = cw1[oc, 0, du + 1, dx + 1]
    return m


def _c1d(cw1, dx, variant):
    m = np.zeros((47, 88), np.float32)
    off = 1 if variant == "B" else 0
    for up in range(22):
        for du in (-1, 0, 1):
            p = 2 * up + du + 1 + off
            if 0 <= p < 47:
                for oc in range(4):
                    m[p, oc * 22 + up] = cw1[oc, 0, du + 1, dx + 1]
    return m


def _band(cw, cb, dx, off, n_in, n_out, in_stride2, with_bias):
    """Generic banded lhsT for conv2/3/4.

    cw: [OC, IC, 3, 3]; input rows u in [0, n_in) per ic; output rows
    j in [0, n_out) per oc; tap u = (2j if in_stride2 else j) + du + off.
    Returns [IC*n_in + 1, OC*n_out] with the ones-row bias (bias + 1 -
    sum of present taps over all dx) when with_bias.
    """
    OC, IC = cw.shape[0], cw.shape[1]
    K = IC * n_in
    m = np.zeros((K + (1 if with_bias is not None else 0), OC * n_out),
                 np.float32)
    for j in range(n_out):
        base = 2 * j if in_stride2 else j
        for du in (-1, 0, 1):
            u = base + du + off
            if 0 <= u < n_in:
                for ic in range(IC):
                    for oc in range(OC):
                        m[ic * n_in + u, oc * n_out + j] = cw[oc, ic, du + 1,
                                                              dx + 1]
    if with_bias:
        for j in range(n_out):
            base = 2 * j if in_stride2 else j
            dus = [du for du in (-1, 0, 1) if 0 <= base + du + off < n_in]
            for oc in range(OC):
                pres = cw[oc, :, [du + 1 for du in dus], :].sum()
                m[K, oc * n_out + j] = 1.0 + cb[oc] - pres
    return m


def _edge_bias(cw, cb, off, n_in, n_out, in_stride2):
    """Per-column constant c = cb - sum(present taps) for the given variant
    offset; [OC*n_out] vector."""
    OC = cw.shape[0]
    out = np.zeros(OC * n_out, np.float32)
    for j in range(n_out):
        base = 2 * j if in_stride2 else j
        dus = [du for du in (-1, 0, 1) if 0 <= base + du + off < n_in]
        for oc in range(OC):
            out[oc * n_out + j] = cb[oc] - cw[oc, :, [du + 1 for du in dus],
                                              :].sum()
    return out


def _prep_shared(I):
    cw1, cw2 = np.asarray(I["cw1"], np.float32), np.asarray(I["cw2"], np.float32)
    cw3, cw4 = np.asarray(I["cw3"], np.float32), np.asarray(I["cw4"], np.float32)
    cb1, cb2 = np.asarray(I["cb1"], np.float32), np.asarray(I["cb2"], np.float32)
    cb3, cb4 = np.asarray(I["cb3"], np.float32), np.asarray(I["cb4"], np.float32)

    d = {}
    for i, dx in enumerate((-1, 0, 1)):
        wb = dx == 0
        for v, off in (("A", 0), ("B", 1)):
            d[f"c1s_{i}{v}"] = _c1s(cw1, dx, v)
            d[f"c1d_{i}{v}"] = _c1d(cw1, dx, v)
            d[f"c2_{i}{v}"] = _band(cw2, cb2, dx, off, 22, 11, True, wb)
            d[f"c3_{i}{v}"] = _band(cw3, None, dx, off, 11, 5, True, None)
        d[f"c4_{i}"] = _band(cw4, None, dx, 0, 5, 5, False, None)
    d["b1"] = np.repeat(cb1, 32).reshape(128, 1)
    d["b1p1"] = d["b1"] + 1.0
    d["b1d"] = np.repeat(cb1, 22).reshape(88, 1)
    d["b1dp1"] = d["b1d"] + 1.0
    # conv3: variant B has no dropped taps; cB = cb3 - sum(all taps)
    cB3 = _edge_bias(cw3, cb3, 1, 11, 5, True)
    cA3 = _edge_bias(cw3, cb3, 0, 11, 5, True)
    d["b3"] = cB3.reshape(80, 1)
    d["b3p1"] = d["b3"] + 1.0
    d["d3"] = (cA3 - cB3).reshape(80, 1)
    c4b = _edge_bias(cw4, cb4, 0, 5, 5, False)
    d["b4"] = c4b.reshape(80, 1)
    d["b4p1"] = d["b4"] + 1.0

    fc1w = np.asarray(I["fc1w"], np.float32)
    fc1b = np.asarray(I["fc1b"], np.float32)
    for x4 in range(5):
        m = np.zeros((81, 128), np.float32)
        for oc in range(16):
            for u in range(5):
                m[oc * 5 + u, :] = fc1w[:, oc * 25 + u * 5 + x4]
        if x4 == 0:
            m[80, :] = 1.0 + fc1b - fc1w.sum(1)
        d[f"fc1_{x4}"] = m

    fc2w = np.asarray(I["fc2w"], np.float32)
    d["fc2T"] = fc2w.T.copy()
    d["bfc2"] = (np.asarray(I["fc2b"], np.float32) - fc2w.sum(1)).reshape(40, 1)

    rt = np.sqrt(np.float32(40.0))
    wq, wk, wv = (np.asarray(I[k], np.float32) for k in ("wq", "wk", "wv"))
    d["wqT"] = np.concatenate(
        [wq.T, ((I["bq"] - wq.sum(1)) / rt)[None]], 0)
    d["wqT"][:40] /= rt
    d["wkT"] = np.concatenate([wk.T, (I["bk"] - wk.sum(1))[None]], 0)
    d["wvT"] = np.concatenate([wv.T, (I["bv"] - wv.sum(1))[None]], 0)

    h1 = np.zeros((41, 32), np.float32)
    h2 = np.zeros((31, 3), np.float32)
    for h, (w1k, b1k, w2k, b2k) in enumerate(
        [("flw1", "flb1", "flw2", "flb2"), ("stw1", "stb1", "stw2", "stb2"),
         ("edw1", "edb1", "edw2", "edb2")]):
        h1[:40, h * 10:h * 10 + 10] = I[w1k].T
        h1[40, h * 10:h * 10 + 10] = np.asarray(I[b1k]) + 1.0
        h2[h * 10:h * 10 + 10, h] = I[w2k][0]
        h2[30, h] = I[b2k][0] - np.asarray(I[w2k][0]).sum()
    d["h1"] = h1
    d["h2"] = h2
    d["eye"] = np.eye(128, dtype=np.float32)

    wpack = np.zeros((128, W_NCOL), np.float32)
    for nm, (p, w, c0) in W_OFF.items():
        a = d[nm]
        assert a.shape == (p, w), (nm, a.shape, (p, w))
        wpack[:p, c0:c0 + w] = a
    return wpack


def _prep_core(I, c, wpack):
    b, l0 = c // 2, (c % 2) * W
    z = np.asarray(I["z"], np.float32)[b, 0]
    dur = int(np.asarray(I["dur"]).reshape(-1)[b])

    # zp: row i <-> global row l0-23+i, col j <-> x j-1 (zero padded)
    zp = np.zeros((ZPR, 82), np.float32)
    g0, g1 = max(l0 - 23, 0), min(l0 + ZPR - 23, L)
    zp[g0 - (l0 - 23):g1 - (l0 - 23), 1:81] = z[g0:g1]

    # direct windows, host-materialized: wind[u+1, wd, x+1]
    wind = np.zeros((47, 44, 82), np.float32)
    for wd in range(44):
        w = wd if wd < 22 else wd + WI
        l = l0 + w
        end = min(l + 23, L)
        start = max(l - 22, 0)
        n = end - start
        wind[U - n + 1:U + 1, wd, 1:81] = z[start:end]

    # row-0 fixup, all windows (stored shifted: elu+1); ch 4 = ones row
    cw1 = np.asarray(I["cw1"], np.float32)
    cb1 = np.asarray(I["cb1"], np.float32)
    zp2 = np.zeros((902, 82), np.float32)
    zp2[1:901, 1:81] = z
    c1f = np.zeros((4, 900, 80), np.float32)
    for du in range(3):
        for dx in range(3):
            c1f += cw1[:, 0, du, dx, None, None] * zp2[du:du + 900,
                                                       dx:dx + 80]
    sh = np.zeros((4, 900, 80), np.float32)
    for dx in range(3):
        sh += cw1[:, 0, 0, dx, None, None] * zp2[0:900, dx:dx + 80]
    ss = l0 + np.arange(22, 428) - 22
    r0 = c1f[:, ss, :] - sh[:, ss, :]
    r1 = c1f[:, ss + 1, :]
    row = np.maximum(r0, r1).reshape(4, WI, 40, 2).max(-1)
    row = row + cb1[:, None, None]
    p1fx = np.ones((5, W, 40), np.float32)
    p1fx[:4, 22:428] = (np.where(row > 0, row,
                                 np.exp(np.minimum(row, 0)) - 1) + 1)
    # direct windows: first pooled row from the materialized window
    for wd in range(44):
        w = wd if wd < 22 else wd + WI
        cwin = np.zeros((4, 2, 80), np.float32)
        for du in range(3):
            for dx in range(3):
                for u in range(2):
                    cwin[:, u, :] += (cw1[:, 0, du, dx, None]
                                      * wind[u + du, wd, dx:dx + 80])
        rowd = cwin.max(1).reshape(4, 40, 2).max(-1) + cb1[:, None]
        p1fx[:4, w] = (np.where(rowd > 0, rowd,
                                np.exp(np.minimum(rowd, 0)) - 1) + 1)

    mv = (np.arange(L) < dur).astype(np.float32)
    cpack = np.zeros((41, C_NCOL), np.float32)
    cpack[:, C_MVB:C_MVB + L] = mv[None, :]
    cpack[:, C_MVL:C_MVL + W] = mv[None, l0:l0 + W]
    t = np.asarray(I["targets_onehot"], np.float32)[b]
    embw = np.asarray(I["embw"], np.float32)
    emb_aug = np.zeros((41, 40), np.float32)
    emb_aug[:40] = embw[:, :40].T
    emb_aug[40] = np.asarray(I["embb"], np.float32) + embw[:, 40:] @ t + 1.0
    cpack[:, C_EMB:C_EMB + 40] = emb_aug

    m = {"zp": zp, "wind": wind, "p1fx": p1fx, "wpack": wpack, "cpack": cpack}
    return {k: np.ascontiguousarray(v, np.float32) for k, v in m.items()}


# ---------------------------------------------------------------------------
# device program
# ---------------------------------------------------------------------------

def _build_program():
    nc = bacc.Bacc("TRN2", target_bir_lowering=False, debug=False,
                   num_devices=NC)

    zp_d = nc.dram_tensor("zp", [ZPR, 82], F32, kind="ExternalInput").ap()
    wind_d = nc.dram_tensor("wind", [47, 44, 82], F32,
                            kind="ExternalInput").ap()
    p1fx_d = nc.dram_tensor("p1fx", [5, W, 40], F32,
                            kind="ExternalInput").ap()
    wpack_d = nc.dram_tensor("wpack", [128, W_NCOL], F32,
                             kind="ExternalInput").ap()
    cpack_d = nc.dram_tensor("cpack", [41, C_NCOL], F32,
                             kind="ExternalInput").ap()
    out_d = nc.dram_tensor("out", [3, W], F32, kind="ExternalOutput").ap()

    zt_loc = nc.dram_tensor("zt_loc", [40 * W], F32)
    zt_full = nc.dram_tensor("zt_full", [2, 40 * W], F32)

    import contextlib
    with tile.TileContext(nc) as tc, contextlib.ExitStack() as ctx:
        wp = ctx.enter_context(tc.tile_pool(name="w", bufs=1))
        sb = ctx.enter_context(tc.tile_pool(name="sb", bufs=2))
        pr = ctx.enter_context(tc.tile_pool(name="pr", bufs=1))
        ps = ctx.enter_context(tc.tile_pool(name="ps", bufs=2, space="PSUM"))
        ps1 = ctx.enter_context(tc.tile_pool(name="ps1", bufs=1,
                                             space="PSUM"))
        dr = ctx.enter_context(tc.tile_pool(name="dr", bufs=3, space="DRAM"))
        sb3 = ctx.enter_context(tc.tile_pool(name="sb3", bufs=3))

        wt = wp.tile([128, W_NCOL], F32R, tag="wpack")
        nc.sync.dma_start(wt[:, 0:W_SPLIT1],
                          wpack_d[:, 0:W_SPLIT1].bitcast(F32R))
        nc.sync.dma_start(wt[:, W_SPLIT1:W_SPLIT2],
                          wpack_d[:, W_SPLIT1:W_SPLIT2].bitcast(F32R))
        nc.sync.dma_start(wt[:, W_SPLIT2:],
                          wpack_d[:, W_SPLIT2:].bitcast(F32R))
        ct = wp.tile([41, C_NCOL], F32R, tag="cpack")
        nc.sync.dma_start(ct[:], cpack_d.bitcast(F32R))

        def ws(nm):
            p, w, c0 = W_OFF[nm]
            return wt[0:p, c0:c0 + w]

        neg1 = wp.tile([128, 1], F32, tag="neg1")
        nc.gpsimd.memset(neg1[:], -1.0)

        def wb(nm, P):
            return ws(nm)[0:P, :].bitcast(F32)

        feat = pr.tile([81, W, 5], F32R, tag="feat")
        nc.gpsimd.memset(feat[:].bitcast(F32), 1.0)

        def block_ones(nn, pA, pB, out_ap, tagsuf):
            """conv2 style (bias in ones row): t1=max(pA,pB); t2=pairmax;
            e=exp(t2-1); out=max(min(e,1),t2).  HW allows only one PSUM
            operand per instruction, so pA is staged through SBUF."""
            P = pA.shape[0]
            sA = sb.tile([128, 480], F32, tag="b_sA" + tagsuf)
            nc.scalar.activation(sA[0:P, 0:nn], pA, AF.Identity)
            t1 = sb.tile([128, 480], F32, tag="b_t1" + tagsuf)
            nc.vector.tensor_tensor(t1[0:P, 0:nn], sA[0:P, 0:nn], pB, OP.max)
            t2 = sb.tile([128, 240], F32, tag="b_t2" + tagsuf)
            i3 = t1[0:P, 0:nn].rearrange("p (n two) -> p n two", two=2)
            nc.vector.tensor_tensor(t2[0:P, 0:nn // 2].unsqueeze(-1),
                                    i3[:, :, 0:1], i3[:, :, 1:2], OP.max)
            e = sb.tile([128, 240], F32, tag="b_e" + tagsuf)
            nc.scalar.activation(e[0:P, 0:nn // 2], t2[0:P, 0:nn // 2],
                                 AF.Exp, bias=neg1[0:P, :])
            nc.vector.scalar_tensor_tensor(
                out_ap, e[0:P, 0:nn // 2], 1.0, t2[0:P, 0:nn // 2],
                OP.min, OP.max)

        def block_bias(nn, pA, pB, out_ap, tagsuf, b_ap, bp1_ap, d_ap=None):
            """bias-free matmuls: t1=max(pA+d,pB) (d=cA-cB); t2=pairmax;
            e=exp(t2+b); e2=min(e,1); out=max(t2+b+1, e2)."""
            P = pA.shape[0]
            sA = sb.tile([128, 480], F32, tag="b_sA" + tagsuf)
            nc.scalar.activation(sA[0:P, 0:nn], pA, AF.Identity,
                                 bias=(d_ap if d_ap is not None else 0.0))
            t1 = sb.tile([128, 480], F32, tag="b_t1" + tagsuf)
            nc.vector.tensor_tensor(t1[0:P, 0:nn], sA[0:P, 0:nn], pB, OP.max)
            t2 = sb.tile([128, 240], F32, tag="b_t2" + tagsuf)
            i3 = t1[0:P, 0:nn].rearrange("p (n two) -> p n two", two=2)
            nc.vector.tensor_tensor(t2[0:P, 0:nn // 2].unsqueeze(-1),
                                    i3[:, :, 0:1], i3[:, :, 1:2], OP.max)
            e = sb.tile([128, 240], F32, tag="b_e" + tagsuf)
            nc.scalar.activation(e[0:P, 0:nn // 2], t2[0:P, 0:nn // 2],
                                 AF.Exp, bias=b_ap)
            e2 = sb.tile([128, 240], F32, tag="b_e2" + tagsuf)
            nc.vector.tensor_scalar_min(e2[0:P, 0:nn // 2],
                                        e[0:P, 0:nn // 2], 1.0)
            nc.vector.scalar_tensor_tensor(
                out_ap, t2[0:P, 0:nn // 2], bp1_ap, e2[0:P, 0:nn // 2],
                OP.add, OP.max)

        # ============ stage 1: conv stack, super-chunks of 60 windows ======
        # one conv1 128-row block serves a 60-window gather (rows <= 102).
        # Software-pipelined: the conv1 + store/gather DMA chain for
        # super-chunk si+1 is emitted before conv2-4 of si, so the ~12us
        # DMA chain overlaps compute instead of idling the PE.
        SCS = [(60 * k, 60) for k in range(7)] + [(420, 30)]

        def conv1_chain(si):
            wa, ww = SCS[si]
            # --- conv1 shared over the super-chunk's 128-row block ---
            zrows = sb3.tile([35, 4, 82], F32R, tag="zrows")
            nc.scalar.dma_start(
                zrows[:],
                bass.AP(zp_d.tensor, wa * 82,
                        [[82, 35], [32 * 82, 4], [1, 82]]).bitcast(F32R))
            pA = ps1.tile([128, 4, 80], F32, tag="cA")
            pB = ps1.tile([128, 4, 80], F32, tag="cB")
            for i in range(3):
                nc.tensor.matmul(pA[:], ws(f"c1s_{i}A"),
                                 zrows[:, :, i:i + 80],
                                 start=(i == 0), stop=(i == 2))
            for i in range(3):
                nc.tensor.matmul(pB[:], ws(f"c1s_{i}B"),
                                 zrows[:, :, i:i + 80],
                                 start=(i == 0), stop=(i == 2))
            pech = sb3.tile([128, 4, 40], F32, tag="pech")
            block_bias(320, pA[:].rearrange("p a b -> p (a b)"),
                       pB[:].rearrange("p a b -> p (a b)"),
                       pech[:].rearrange("p a b -> p (a b)"), "ch",
                       wb("b1", 128), wb("b1p1", 128))

            # --- row128-linear staging in DRAM (row = 32*sub + j) ---
            p1e = dr.tile([4, 128, 40], F32R, tag="p1e")
            for oc in range(4):
                nc.sync.dma_start(
                    bass.AP(p1e[:].tensor, oc * 5120,
                            [[40, 32], [1280, 4], [1, 40]]),
                    bass.AP(pech[:].tensor, oc * 5120,
                            [[160, 32], [40, 4], [1, 40]]).bitcast(F32R))

            # --- conv2 input gather (windows overlap => per-window rows) ---
            c2p = ww * 42
            c2in = sb3.tile([89, ww, 42], F32R, tag="c2in")
            nc.gpsimd.memset(c2in[:, :, 0:1].bitcast(F32), 1.0)
            nc.gpsimd.memset(c2in[:, :, 41:42].bitcast(F32), 1.0)
            ia, ib = max(wa, 22), min(wa + ww, 428)
            n = ib - ia
            for ic in range(4):
                nc.sync.dma_start(
                    bass.AP(c2in[:].tensor, ic * 22 * c2p + (ia - wa) * 42 + 1,
                            [[c2p, 22], [42, n], [1, 40]]),
                    bass.AP(p1e[:].tensor, ic * 5120 + (ia - wa) * 40,
                            [[80, 22], [40, n], [1, 40]]))

            # --- direct (head/tail) windows ---
            if si == 0 or si == 7:
                for gi, (s0, nd) in enumerate(((0, 6), (6, 6), (12, 6),
                                               (18, 4))):
                    wd0 = s0 + (0 if si == 0 else 22)
                    wloc = wd0 if si == 0 else wd0 + WI - 420
                    wint = sb.tile([47, 6, 82], F32R, tag="wint")
                    nc.scalar.dma_start(
                        wint[:, 0:nd, :],
                        wind_d[:, wd0:wd0 + nd, :].bitcast(F32R))
                    dA = ps1.tile([88, 6, 80], F32, tag="cA")
                    dB = ps1.tile([88, 6, 80], F32, tag="cB")
                    for i in range(3):
                        nc.tensor.matmul(dA[:, 0:nd, :], ws(f"c1d_{i}A"),
                                         wint[:, 0:nd, i:i + 80],
                                         start=(i == 0), stop=(i == 2))
                    for i in range(3):
                        nc.tensor.matmul(dB[:, 0:nd, :], ws(f"c1d_{i}B"),
                                         wint[:, 0:nd, i:i + 80],
                                         start=(i == 0), stop=(i == 2))
                    dbuf = sb.tile([88, 6, 40], F32, tag="dbuf")
                    block_bias(
                        nd * 80,
                        dA[:].rearrange("p a b -> p (a b)")[:, 0:nd * 80],
                        dB[:].rearrange("p a b -> p (a b)")[:, 0:nd * 80],
                        dbuf[:].rearrange("p a b -> p (a b)")[:, 0:nd * 40],
                        "ch", wb("b1d", 88), wb("b1dp1", 88))
                    nc.sync.dma_start(
                        c2in[0:88, wloc:wloc + nd, 1:41],
                        dbuf[:, 0:nd, :].bitcast(F32R))

            # --- u=0 fixup + ones row for every window of the super-chunk ---
            nc.sync.dma_start(
                bass.AP(c2in[:].tensor, 1,
                        [[22 * c2p, 5], [42, ww], [1, 40]]),
                bass.AP(p1fx_d.tensor, wa * 40,
                        [[W * 40, 5], [40, ww], [1, 40]]).bitcast(F32R))

            return c2in

        def conv234(si, c2in):
            wa, ww = SCS[si]
            # --- conv2: N-chunks of 10 windows ---
            t2e = sb.tile([88, ww, 22], F32R, tag="t2e")
            nc.gpsimd.memset(t2e[:, :, 0:1].bitcast(F32), 1.0)
            nc.gpsimd.memset(t2e[:, :, 21:22].bitcast(F32), 1.0)
            for k in range(ww // 10):
                na = k * 10
                pA2 = ps.tile([88, 10, 40], F32, tag="pA")
                pB2 = ps.tile([88, 10, 40], F32, tag="pB")
                for i in range(3):
                    rhs = c2in[:, na:na + 10, i:i + 40]
                    nc.tensor.matmul(pA2[:], ws(f"c2_{i}A"), rhs,
                                     start=(i == 0), stop=(i == 2))
                for i in range(3):
                    rhs = c2in[:, na:na + 10, i:i + 40]
                    nc.tensor.matmul(pB2[:], ws(f"c2_{i}B"), rhs,
                                     start=(i == 0), stop=(i == 2))
                block_ones(400, pA2[:].rearrange("p a b -> p (a b)"),
                           pB2[:].rearrange("p a b -> p (a b)"),
                           t2e[:, na:na + 10, 1:21], "c2")

            # --- conv3: N-chunks of 15 ---
            t3e = sb.tile([80, ww, 12], F32R, tag="t3e")
            nc.gpsimd.memset(t3e[:, :, 0:1].bitcast(F32), 1.0)
            nc.gpsimd.memset(t3e[:, :, 11:12].bitcast(F32), 1.0)
            for k in range(ww // 15):
                na = k * 15
                pA3 = ps.tile([80, 15, 20], F32, tag="pA")
                pB3 = ps.tile([80, 15, 20], F32, tag="pB")
                for i in range(3):
                    rhs = t2e[:, na:na + 15, i:i + 20]
                    nc.tensor.matmul(pA3[:], ws(f"c3_{i}A"), rhs,
                                     start=(i == 0), stop=(i == 2))
                for i in range(3):
                    rhs = t2e[:, na:na + 15, i:i + 20]
                    nc.tensor.matmul(pB3[:], ws(f"c3_{i}B"), rhs,
                                     start=(i == 0), stop=(i == 2))
                block_bias(300, pA3[:].rearrange("p a b -> p (a b)"),
                           pB3[:].rearrange("p a b -> p (a b)"),
                           t3e[:, na:na + 15, 1:11], "c3",
                           wb("b3", 80), wb("b3p1", 80), wb("d3", 80))

            # --- conv4 (pool 1x2 only), groups of 30 ---
            for k in range(ww // 30):
                na = k * 30
                pC4 = ps1.tile([80, 30, 10], F32, tag="pC")
                for i in range(3):
                    nc.tensor.matmul(pC4[:], ws(f"c4_{i}"),
                                     t3e[:, na:na + 30, i:i + 10],
                                     start=(i == 0), stop=(i == 2))
                s4 = sb.tile([128, 480], F32, tag="b_sAc1")
                nc.scalar.activation(s4[0:80, 0:300],
                                     pC4[:].rearrange("p a b -> p (a b)"),
                                     AF.Identity)
                t2c = sb.tile([128, 240], F32, tag="b_t2c1")
                i3 = s4[0:80, 0:300].rearrange("p (n two) -> p n two", two=2)
                nc.vector.tensor_tensor(t2c[0:80, 0:150].unsqueeze(-1),
                                        i3[:, :, 0:1], i3[:, :, 1:2], OP.max)
                e4 = sb.tile([128, 240], F32, tag="b_ec1")
                nc.scalar.activation(e4[0:80, 0:150], t2c[0:80, 0:150],
                                     AF.Exp, bias=wb("b4", 80))
                e42 = sb.tile([128, 240], F32, tag="b_e2c1")
                nc.vector.tensor_scalar_min(e42[0:80, 0:150],
                                            e4[0:80, 0:150], 1.0)
                nc.vector.scalar_tensor_tensor(
                    feat[0:80, wa + na:wa + na + 30, :].rearrange(
                        "p a b -> p (a b)"),
                    t2c[0:80, 0:150], wb("b4p1", 80), e42[0:80, 0:150],
                    OP.add, OP.max)

        from collections import deque
        pend = deque([conv1_chain(0), conv1_chain(1)])
        for si in range(len(SCS)):
            if si + 2 < len(SCS):
                pend.append(conv1_chain(si + 2))
            conv234(si, pend.popleft())

        # ============ stage 3: fc1/fc2/emb -> z_t ==========================
        f1 = ps.tile([128, W], F32, tag="pA")
        for x4 in range(5):
            nc.tensor.matmul(f1[:], ws(f"fc1_{x4}"), feat[:, :, x4:x4 + 1],
                             start=(x4 == 0), stop=(x4 == 4))
        ef = sb.tile([128, W], F32, tag="ef")
        nc.scalar.activation(ef[:], f1[:], AF.Exp, bias=neg1[:])
        fc1e = pr.tile([128, W], F32R, tag="fc1e")
        nc.vector.scalar_tensor_tensor(fc1e[:], ef[:], 1.0, f1[:],
                                       OP.min, OP.max)

        zp0 = ps.tile([40, W], F32, tag="pB")
        nc.tensor.matmul(zp0[:], ws("fc2T"), fc1e[:], start=True, stop=True)
        zp0s = pr.tile([41, W], F32R, tag="zp0s")
        nc.gpsimd.memset(zp0s[:].bitcast(F32), 1.0)
        nc.scalar.activation(zp0s[0:40, :], zp0[:], AF.Identity,
                             bias=ws("bfc2").bitcast(F32))

        ztp = ps.tile([40, W], F32, tag="pA")
        nc.tensor.matmul(ztp[:], ct[:, C_EMB:C_EMB + 40], zp0s[:],
                         start=True, stop=True)
        ez = sb.tile([40, W], F32, tag="ef")
        nc.scalar.activation(ez[:], ztp[:], AF.Exp, bias=neg1[0:40, :])
        zt = pr.tile([41, W], F32R, tag="zt")
        nc.gpsimd.memset(zt[:].bitcast(F32), 1.0)
        nc.vector.scalar_tensor_tensor(zt[0:40, :], ez[:], 1.0, ztp[:],
                                       OP.min, OP.max)

        # ============ stage 4: AllGather z_t (shifted) =====================
        nc.sync.dma_start(zt_loc.ap().rearrange("(p f) -> p f", p=40),
                          zt[0:40, :].bitcast(F32))
        nc.gpsimd.collective_compute(
            "AllGather", OP.bypass,
            replica_groups=[[0, 1], [2, 3], [4, 5], [6, 7]],
            ins=[zt_loc.ap()], outs=[zt_full.ap()])
        zta = pr.tile([41, L], F32R, tag="zta")
        nc.gpsimd.memset(zta[:].bitcast(F32), 1.0)
        nc.sync.dma_start(
            zta[0:40, :],
            bass.AP(zt_full.ap().tensor, 0,
                    [[W, 40], [40 * W, 2], [1, W]]).bitcast(F32R))

        # ============ stage 5: attention ===================================
        qp = ps.tile([40, W], F32, tag="pB")
        nc.tensor.matmul(qp[:], ws("wqT"), zt[:], start=True, stop=True)
        q_sb = pr.tile([40, W], F32R, tag="q_sb")
        nc.vector.tensor_tensor(q_sb[:], qp[:],
                                ct[0:40, C_MVL:C_MVL + W], OP.mult)

        k_sb = pr.tile([40, L], F32R, tag="k_sb")
        for h in range(2):
            kp = ps.tile([40, W], F32, tag="pA")
            nc.tensor.matmul(kp[:], ws("wkT"), zta[:, h * W:(h + 1) * W],
                             start=True, stop=True)
            nc.vector.tensor_tensor(k_sb[:, h * W:(h + 1) * W], kp[:],
                                    ct[0:40, C_MVB + h * W:C_MVB + (h + 1) * W],
                                    OP.mult)

        vps = ps1.tile([113, 8, 40], F32, tag="pC")
        m0 = 0
        for mg in range(8):
            nc.tensor.matmul(vps[0:MG[mg], mg:mg + 1, :],
                             zta[:, m0:m0 + MG[mg]], ws("wvT"),
                             start=True, stop=True)
            m0 += MG[mg]
        v_all = pr.tile([113, 8, 40], F32R, tag="v_all")
        nc.scalar.activation(v_all[0:113, 0:7, :], vps[0:113, 0:7, :],
                             AF.Identity)
        nc.scalar.activation(v_all[0:109, 7:8, :], vps[0:109, 7:8, :],
                             AF.Identity)

        ET = pr.tile([113, 8, W], F32R, tag="ET")
        l0g = 0
        for g in range(4):
            lg = LG[g]
            s0 = ps.tile([113, W], F32, tag="pA")
            s1 = ps.tile([113, W], F32, tag="pB")
            nc.tensor.matmul(s0[0:lg, :], q_sb[:, l0g:l0g + lg],
                             k_sb[:, 0:W], start=True, stop=True)
            nc.tensor.matmul(s1[0:lg, :], q_sb[:, l0g:l0g + lg],
                             k_sb[:, W:L], start=True, stop=True)
            E = sb.tile([113, L], F32, tag="E")
            racc = sb.tile([113, 2], F32, tag="racc")
            nc.scalar.activation(E[0:lg, 0:W], s0[0:lg, :], AF.Exp,
                                 accum_out=racc[0:lg, 0:1])
            nc.scalar.activation(E[0:lg, W:L], s1[0:lg, :], AF.Exp,
                                 accum_out=racc[0:lg, 1:2])
            rs = sb.tile([113, 1], F32, tag="rs")
            nc.vector.tensor_tensor(rs[0:lg, :], racc[0:lg, 0:1],
                                    racc[0:lg, 1:2], OP.add)
            rr = sb.tile([113, 1], F32, tag="rr")
            nc.vector.reciprocal(rr[0:lg, :], rs[0:lg, :])
            nc.vector.tensor_scalar_mul(E[0:lg, :], E[0:lg, :], rr[0:lg, 0:1])
            for half in range(2):
                tr = ps1.tile([113, 4, 113], F32, tag="pC")
                for t in range(4):
                    mg = half * 4 + t
                    m0 = 113 * mg
                    nc.tensor.transpose(tr[0:MG[mg], t:t + 1, 0:lg],
                                        E[0:lg, m0:m0 + MG[mg]],
                                        ws("eye")[0:lg, 0:lg].bitcast(F32))
                if half == 0:
                    nc.scalar.activation(
                        ET[0:113, 0:4, l0g:l0g + lg],
                        tr[0:113, 0:4, 0:lg], AF.Identity)
                else:
                    nc.scalar.activation(
                        ET[0:113, 4:7, l0g:l0g + lg],
                        tr[0:113, 0:3, 0:lg], AF.Identity)
                    nc.scalar.activation(
                        ET[0:109, 7:8, l0g:l0g + lg],
                        tr[0:109, 3:4, 0:lg], AF.Identity)
            l0g += lg

        xp_ = ps1.tile([40, W], F32, tag="pC")
        for mg in range(8):
            nc.tensor.matmul(xp_[:], v_all[0:MG[mg], mg:mg + 1, :],
                             ET[0:MG[mg], mg:mg + 1, :], start=(mg == 0),
                             stop=(mg == 7))
        x_aug = pr.tile([41, W], F32R, tag="x_aug")
        nc.gpsimd.memset(x_aug[:].bitcast(F32), 1.0)
        nc.vector.scalar_tensor_tensor(x_aug[0:40, :], zt[0:40, :], -1.0,
                                       xp_[:], OP.add, OP.add)

        # ============ stage 6: heads =======================================
        h1p = ps.tile([32, W], F32, tag="pA")
        nc.tensor.matmul(h1p[:], ws("h1"), x_aug[:], start=True, stop=True)
        eh = sb.tile([32, W], F32, tag="ef")
        nc.scalar.activation(eh[0:30, :], h1p[0:30, :], AF.Exp,
                             bias=neg1[0:30, :])
        h1e = pr.tile([31, W], F32R, tag="h1e")
        nc.gpsimd.memset(h1e[:].bitcast(F32), 1.0)
        nc.vector.scalar_tensor_tensor(h1e[0:30, :], eh[0:30, :], 1.0,
                                       h1p[0:30, :], OP.min, OP.max)
        o3 = ps.tile([3, W], F32, tag="pB")
        nc.tensor.matmul(o3[:], ws("h2"), h1e[:], start=True, stop=True)
        osb = sb.tile([3, W], F32, tag="osb")
        nc.vector.tensor_copy(osb[:], o3[:])
        nc.sync.dma_start(out_d, osb[:])

    nc.compile()
    return nc


def _get_program():
    if "nc" not in _CACHE:
        _CACHE["nc"] = _build_program()
    return _CACHE["nc"]


def kernel(**inputs):
    I = {k: np.asarray(v) for k, v in inputs.items()}
    nc = _get_program()
    wpack = _prep_shared(I)
    in_maps = [_prep_core(I, c, wpack) for c in range(NC)]
    res = run_bass_kernel_spmd(nc, in_maps, list(range(NC))).results
    fl = np.zeros((4, L), np.float32)
    st = np.zeros((4, L), np.float32)
    ed = np.zeros((4, L), np.float32)
    for c in range(NC):
        b, l0 = c // 2, (c % 2) * W
        o = res[c]["out"]
        fl[b, l0:l0 + W] = o[0]
        st[b, l0:l0 + W] = o[1]
        ed[b, l0:l0 + W] = o[2]
    return fl, st, ed
